# revision 44
# baseline (speedup 1.0000x reference)
"""Two-layer GATv2 (heads=1) on 8 Trainium2 NeuronCores.

Sharding: nodes partitioned by dst across 8 cores (6250 each), dst-blocks of
125 nodes. Edges of a block are sorted lo/hi by src (split at 32768 so
indices fit int16) and processed in groups of 128.

Per layer:
  - matmul phase: xl = x@[Wl | 0.2*Wl@att], xr likewise (PE, bf16); xl rows
    AllGathered into a [N, T] row table (T = 256/128 cols, 512/256B rows).
  - edge phase per dst-block:
      xlg  <- batched dma_gather of xl rows (two gathers: lo/hi table halves)
      psum <- S_T_aug.T @ xr_aug  (one-hot dst select + We*ea + const row)
            + I.T @ xlg           (adds xl[src]; col F accumulates 0.2*att.v)
      r    <- Relu(0.8*psum) on ACT      (lrelu(v) = 0.2v + relu(0.8v))
      logit<- ttr(r, att, mult, add, init=psum_col_F)  -> 0.2*att.v + att.r
      w    <- Exp(logit)
      sw   <- (iota==dst)*w (bf16 one-hot), U += sw.T@xlg, s += sw.T@1 (PE)
      h    <- elu(U/s + b) (+1 folded into next layer's const row)
Weights replicated; layer-2 constant correction (elu's -1) folded into ST
row 126 x xr row 126.
"""
import sys

sys.path.insert(0, '/opt/trn_rl_repo')

import numpy as np
import ml_dtypes
from contextlib import ExitStack

import concourse.bass as bass
import concourse.tile as tile
import concourse.bacc as bacc
from concourse import mybir

BF16 = ml_dtypes.bfloat16
F32 = mybir.dt.float32
BF = mybir.dt.bfloat16
I16 = mybir.dt.int16
AF = mybir.ActivationFunctionType
ALU = mybir.AluOpType

NEG_SLOPE = 0.2

# Problem geometry (nn_Affinity_GAT_75557064671579)
N = 50000
E = 800000
F_IN = 128
H = 128
OUT = 64
NC_N = 8
NLOC = N // NC_N        # 6250 nodes per core
BLK = 125               # dst nodes per block
NB = NLOC // BLK        # 50 blocks per core
LO = 32768              # src table split (int16 index limit)
PAD_DST = 1000.0

LAST_EXEC_NS = [None]
LAST_RESULTS = [None]
DEBUG = False
DEBUG_LAYER = 1
NQUEUES = 4     # SWDGE queues used for gathers
MAXG = 8        # groups per gather instruction (1024 idxs; ring cap ~65/dma)
USE_TTR = False  # InstTensorTensorReduce wedges real HW; use stt + av add


def prep_inputs(x, edge_index, edge_attr):
    """Host-side graph prep: self loops, dst-shard, block layout, lo/hi
    src split, gather indices, one-hot S_T_aug."""
    src = edge_index[0].astype(np.int64)
    dst = edge_index[1].astype(np.int64)
    ea = edge_attr[:, 0].astype(np.float32)

    # add_self_loops with fill_value='mean' over incoming edges
    cnt = np.bincount(dst, minlength=N).astype(np.float32)
    asum = np.bincount(dst, weights=ea, minlength=N).astype(np.float32)
    loop_attr = (asum / np.maximum(cnt, 1.0)).astype(np.float32)
    src_all = np.concatenate([src, np.arange(N, dtype=np.int64)])
    dst_all = np.concatenate([dst, np.arange(N, dtype=np.int64)])
    ea_all = np.concatenate([ea, loop_attr])

    core_of = dst_all // NLOC
    percore = []
    nlo = np.zeros((NC_N, NB), np.int64)
    nhi = np.zeros((NC_N, NB), np.int64)
    for c in range(NC_N):
        m = core_of == c
        s_c = src_all[m]
        d_c = dst_all[m] - c * NLOC
        a_c = ea_all[m]
        b_c = d_c // BLK
        islo = s_c < LO
        key = b_c * 2 + (~islo).astype(np.int64)
        order = np.argsort(key, kind='stable')
        s_c, d_c, a_c, b_c, islo = (s_c[order], d_c[order], a_c[order],
                                    b_c[order], islo[order])
        percore.append((s_c, d_c, a_c, b_c, islo))
        nlo[c] = np.bincount(b_c[islo], minlength=NB)
        nhi[c] = np.bincount(b_c[~islo], minlength=NB)

    g_lo = ((nlo.max(axis=0) + 127) // 128).astype(np.int64)
    g_hi = ((nhi.max(axis=0) + 127) // 128).astype(np.int64)
    g_all = g_lo + g_hi
    g_start = np.concatenate([[0], np.cumsum(g_all)])
    g_total = int(g_start[-1])

    core_inputs = []
    for c in range(NC_N):
        s_c, d_c, a_c, b_c, islo = percore[c]
        idx16 = np.zeros((128, 8 * g_total), np.int16)
        dstc = np.full((128, g_total), PAD_DST, np.float32)
        ST = np.zeros((128, 128 * g_total), np.float32)

        # slot index per edge: block base + lo slot / hi region slot
        off_l = np.zeros(NB, np.int64)
        off_h = np.zeros(NB, np.int64)
        slot = np.zeros(len(s_c), np.int64)
        # edges are sorted by (block, hi) so within each (block, half)
        # they are consecutive
        for b in range(NB):
            lo_n = int(nlo[c, b])
            hi_n = int(nhi[c, b])
            base = g_start[b] * 128
            sel_lo = (b_c == b) & islo
            sel_hi = (b_c == b) & ~islo
            slot[sel_lo] = base + np.arange(lo_n)
            slot[sel_hi] = base + 128 * g_lo[b] + np.arange(hi_n)

        din = (d_c % BLK).astype(np.int64)
        ST[din, slot] = 1.0
        ST[126, slot] = 1.0
        ST[127, slot] = a_c
        dstc[slot % 128, slot // 128] = din.astype(np.float32)
        # Q7 cpu pair for queue q reads idxs from partitions [32q, 32q+32):
        # replicate the [16, cols] block across all 128 partitions.
        idxval = np.where(islo, s_c, s_c - LO).astype(np.int16)
        idx16[slot % 16, slot // 16] = idxval
        idx16 = np.tile(idx16[0:16], (8, 1))

        xT = np.ascontiguousarray(
            x[c * NLOC:(c + 1) * NLOC].T).astype(BF16)  # [F_IN, NLOC]
        core_inputs.append(dict(
            xT_slice=xT,
            idx_d=idx16,
            dstc_d=dstc,
            ST_d=ST.astype(BF16),
        ))
    meta = (tuple(g_lo.tolist()), tuple(g_hi.tolist()))
    return core_inputs, g_lo, g_hi, g_start, g_total, meta


def build_consts(Wl1, Wr1, We1, att1, b1, Wl2, Wr2, We2, att2, b2):
    att1 = np.asarray(att1, np.float32)
    att2 = np.asarray(att2, np.float32)
    Wl1 = np.asarray(Wl1, np.float32)
    Wr1 = np.asarray(Wr1, np.float32)
    Wl2 = np.asarray(Wl2, np.float32)
    Wr2 = np.asarray(Wr2, np.float32)
    We1 = np.asarray(We1, np.float32).reshape(-1)   # [H]
    We2 = np.asarray(We2, np.float32).reshape(-1)   # [OUT]

    def aug(W, att):
        return np.concatenate([W, NEG_SLOPE * (W @ att)[:, None]],
                              axis=1).astype(BF16)

    c2 = -(Wl2.sum(axis=0) + Wr2.sum(axis=0))       # [OUT]
    we1row = np.concatenate([We1, [NEG_SLOPE * float(We1 @ att1)]])
    we2row = np.concatenate([We2, [NEG_SLOPE * float(We2 @ att2)]])
    c2row = np.concatenate([c2, [NEG_SLOPE * float(c2 @ att2)]])

    return dict(
        Wl1aug=aug(Wl1, att1), Wr1aug=aug(Wr1, att1),
        Wl2aug=aug(Wl2, att2), Wr2aug=aug(Wr2, att2),
        We1rep=np.tile(we1row[None, :], (1, NB)).astype(BF16),
        We2rep=np.tile(we2row[None, :], (1, NB)).astype(BF16),
        Cr2rep=np.tile(c2row[None, :], (1, NB)).astype(BF16),
        att1t=np.tile(att1[None, :], (128, 1)).astype(BF16),
        att2t=np.tile(att2[None, :], (128, 1)).astype(BF16),
        b1t=np.tile(np.asarray(b1, np.float32)[None, :], (128, 1)),
        # U2 accumulates raw h@Wl2 rows (each +colsum(Wl2) vs (h-1)@Wl2);
        # U/s is a convex combination, so subtract it once via the bias.
        b2t=np.tile((np.asarray(b2, np.float32) - Wl2.sum(axis=0))[None, :],
                    (128, 1)),
        iota_row=np.tile(np.arange(BLK, dtype=np.float32)[None, :],
                         (128, 1)).astype(BF16),
        ident=np.eye(128, dtype=np.float32).astype(BF16),
        ones_col=np.ones((128, 1), np.float32).astype(BF16),
    )


def build_program(g_lo, g_hi, g_start, g_total):
    nc = bacc.Bacc("TRN2", target_bir_lowering=False, debug=False,
                   num_devices=NC_N, num_swdge_queues=NQUEUES)

    T1, T2 = 256, 128           # gather table row widths (cols)
    FA1, FA2 = H + 1, OUT + 1   # v columns incl. att.v accumulator
    CH1, CH2 = 3, 7             # psum-bank groups per chunk (<=512 f32)

    # ---- DRAM I/O ----
    xT_d = nc.dram_tensor("xT_slice", [F_IN, NLOC], BF, kind="ExternalInput")
    idx_d = nc.dram_tensor("idx_d", [128, 8 * g_total], I16, kind="ExternalInput")
    dstc_d = nc.dram_tensor("dstc_d", [128, g_total], F32, kind="ExternalInput")
    ST_d = nc.dram_tensor("ST_d", [128, 128 * g_total], BF, kind="ExternalInput")
    Wl1_d = nc.dram_tensor("Wl1aug", [F_IN, FA1], BF, kind="ExternalInput")
    Wr1_d = nc.dram_tensor("Wr1aug", [F_IN, FA1], BF, kind="ExternalInput")
    Wl2_d = nc.dram_tensor("Wl2aug", [H, FA2], BF, kind="ExternalInput")
    Wr2_d = nc.dram_tensor("Wr2aug", [H, FA2], BF, kind="ExternalInput")
    We1_d = nc.dram_tensor("We1rep", [1, NB * FA1], BF, kind="ExternalInput")
    We2_d = nc.dram_tensor("We2rep", [1, NB * FA2], BF, kind="ExternalInput")
    Cr2_d = nc.dram_tensor("Cr2rep", [1, NB * FA2], BF, kind="ExternalInput")
    att1_d = nc.dram_tensor("att1t", [128, H], BF, kind="ExternalInput")
    att2_d = nc.dram_tensor("att2t", [128, OUT], BF, kind="ExternalInput")
    b1_d = nc.dram_tensor("b1t", [128, H], F32, kind="ExternalInput")
    b2_d = nc.dram_tensor("b2t", [128, OUT], F32, kind="ExternalInput")
    iota_d = nc.dram_tensor("iota_row", [128, BLK], BF, kind="ExternalInput")
    ident_d = nc.dram_tensor("ident", [128, 128], BF, kind="ExternalInput")
    ones_d = nc.dram_tensor("ones_col", [128, 1], BF, kind="ExternalInput")
    out_d = nc.dram_tensor("out_slice", [NLOC, OUT], F32, kind="ExternalOutput")

    if DEBUG:
        logit1_dbg = nc.dram_tensor("logit1_dbg", [128, g_total], F32,
                                    kind="ExternalOutput")
        h_dbg = nc.dram_tensor("h_dbg", [NLOC, H], BF, kind="ExternalOutput")
        us_dbg = nc.dram_tensor("us_dbg", [NLOC, H + 1], F32,
                                kind="ExternalOutput")

    ag1_d = nc.dram_tensor("ag1_in", [NLOC, T1], BF)
    xl1_d = nc.dram_tensor("xl1_full", [N, T1], BF, addr_space="Shared")
    ag2_d = nc.dram_tensor("ag2_in", [NLOC, T2], BF)
    xl2_d = nc.dram_tensor("xl2_full", [N, T2], BF, addr_space="Shared")

    qn = [0]

    with tile.TileContext(nc) as tc:
        with ExitStack() as ctx:
            cpool = ctx.enter_context(tc.tile_pool(name="const", bufs=1))
            mm_pool = ctx.enter_context(tc.tile_pool(name="mmx", bufs=3))
            st_pool = ctx.enter_context(tc.tile_pool(name="stp", bufs=3))
            xlg_pool = ctx.enter_context(tc.tile_pool(name="xlg", bufs=3))
            idx_pool = ctx.enter_context(tc.tile_pool(name="idxp", bufs=3))
            sm_pool = ctx.enter_context(tc.tile_pool(name="small", bufs=3))
            r_pool = ctx.enter_context(tc.tile_pool(name="rz", bufs=3))
            scr_pool = ctx.enter_context(tc.tile_pool(name="scr", bufs=4))
            sw_pool = ctx.enter_context(tc.tile_pool(name="sw", bufs=4))
            np_pool = ctx.enter_context(tc.tile_pool(name="nodep", bufs=3))
            ps_mm = ctx.enter_context(tc.tile_pool(name="psmm", bufs=1, space="PSUM"))
            ps_v = ctx.enter_context(tc.tile_pool(name="psv", bufs=2, space="PSUM"))
            ps_u = ctx.enter_context(tc.tile_pool(name="psu", bufs=2, space="PSUM"))
            ps_t = ctx.enter_context(tc.tile_pool(name="pst", bufs=1, space="PSUM"))

            def cload(name, dram, shape, dt=BF):
                t = cpool.tile(shape, dt, tag=name)
                nc.sync.dma_start(out=t[:], in_=dram[:])
                return t

            xT_sb = cload("xT", xT_d, [F_IN, NLOC])
            Wl1_sb = cload("Wl1", Wl1_d, [F_IN, FA1])
            Wr1_sb = cload("Wr1", Wr1_d, [F_IN, FA1])
            Wl2_sb = cload("Wl2", Wl2_d, [H, FA2])
            Wr2_sb = cload("Wr2", Wr2_d, [H, FA2])
            att1_sb = cload("att1", att1_d, [128, H])
            att2_sb = cload("att2", att2_d, [128, OUT])
            b1_sb = cload("b1", b1_d, [128, H], F32)
            b2_sb = cload("b2", b2_d, [128, OUT], F32)
            iota_sb = cload("iota", iota_d, [128, BLK])
            ident_sb = cload("ident", ident_d, [128, 128])
            ones_sb = cload("ones", ones_d, [128, 1])

            hT_sb = cpool.tile([H, NLOC], BF, tag="hT")
            xr1_sb = cpool.tile([128, NB * FA1], BF, tag="xr1")
            xr2_sb = cpool.tile([128, NB * FA2], BF, tag="xr2")
            nc.vector.memset(xr1_sb[96:128, :], 0.0)
            nc.vector.memset(xr2_sb[96:128, :], 0.0)
            nc.sync.dma_start(out=xr1_sb[127:128, :], in_=We1_d[:])
            nc.sync.dma_start(out=xr2_sb[126:127, :], in_=Cr2_d[:])
            nc.sync.dma_start(out=xr2_sb[127:128, :], in_=We2_d[:])

            # tail columns of the gather-table rows: col FA (right after
            # the a_l column) is a constant 1.0 so one U-matmul also
            # accumulates s = sum(w) in psum col F+1; the rest are zero.
            zt = cpool.tile([BLK, T1 - FA1], BF, tag="zt")
            nc.vector.memset(zt[:], 0.0)
            nc.vector.memset(zt[:, 0:1], 1.0)
            for j in range(NB):
                rows = slice(j * BLK, (j + 1) * BLK)
                nc.sync.dma_start(out=ag1_d[rows, FA1:T1],
                                  in_=zt[:, 0:T1 - FA1])
                nc.sync.dma_start(out=ag2_d[rows, FA2:T2],
                                  in_=zt[:, 0:T2 - FA2])

            # ================= layer-1 matmul phase =================
            for j in range(NB):
                lhsT = xT_sb[:, j * BLK:(j + 1) * BLK]
                pa = ps_mm.tile([BLK, FA1], F32, tag="mm")
                nc.tensor.matmul(out=pa[:], lhsT=lhsT, rhs=Wl1_sb[:],
                                 start=True, stop=True)
                agt = mm_pool.tile([BLK, FA1], BF, tag="agt")
                nc.scalar.activation(agt[:], pa[:], AF.Copy)
                nc.sync.dma_start(out=ag1_d[j * BLK:(j + 1) * BLK, 0:FA1],
                                  in_=agt[:])
                pb = ps_mm.tile([BLK, FA1], F32, tag="mm")
                nc.tensor.matmul(out=pb[:], lhsT=lhsT, rhs=Wr1_sb[:],
                                 start=True, stop=True)
                nc.scalar.activation(xr1_sb[0:BLK, j * FA1:(j + 1) * FA1],
                                     pb[:], AF.Copy)

            tc.strict_bb_all_engine_barrier()
            if NC_N == 1:
                nc.sync.dma_start(out=xl1_d[:], in_=ag1_d[:])
            else:
                nc.gpsimd.collective_compute(
                    "AllGather", ALU.bypass,
                    replica_groups=[list(range(NC_N))],
                    ins=[ag1_d[:]], outs=[xl1_d[:]])
            tc.strict_bb_all_engine_barrier()

            def edge_layer(F, FA, T, CH, xl_full, xr_sb, att_sb, b_sb, sink,
                           dbg=False):
                for b in range(NB):
                    GL, GH = int(g_lo[b]), int(g_hi[b])
                    G = GL + GH
                    gs = int(g_start[b])
                    ST_t = st_pool.tile([128, 128 * G], BF, tag="st")
                    nc.sync.dma_start(
                        out=ST_t[:],
                        in_=ST_d[:, 128 * gs:128 * (gs + G)])
                    idx_t = idx_pool.tile([128, 8 * G], I16, tag="idx")
                    nc.sync.dma_start(
                        out=idx_t[:], in_=idx_d[:, 8 * gs:8 * (gs + G)])
                    dst_t = sm_pool.tile([128, G], F32, tag="dst")
                    nc.sync.dma_start(
                        out=dst_t[:], in_=dstc_d[:, gs:gs + G])

                    xlg = xlg_pool.tile([128, G, T], BF, tag="xlg")
                    for base, gn, tab in ((0, GL, xl_full[0:LO, :]),
                                          (GL, GH, xl_full[LO:N, :])):
                        for q0 in range(0, gn, MAXG):
                            n = min(MAXG, gn - q0)
                            o = base + q0
                            nc.gpsimd.dma_gather(
                                out_ap=xlg[:, o:o + n, :], in_ap=tab,
                                idxs_ap=idx_t[:, 8 * o:8 * (o + n)],
                                num_idxs=128 * n, num_idxs_reg=128 * n,
                                elem_size=T, queue_num=qn[0] % NQUEUES)
                            qn[0] += 1

                    logit_t = sm_pool.tile([128, G], F32, tag="logit")
                    if not USE_TTR:
                        lacc_t = sm_pool.tile([128, G], F32, tag="lacc")
                        av_t = sm_pool.tile([128, G], F32, tag="av")
                    xr_b = xr_sb[:, b * FA:(b + 1) * FA]
                    for c0 in range(0, G, CH):
                        ng = min(CH, G - c0)
                        pv = ps_v.tile([128, CH * FA], F32, tag="pv")
                        for gi in range(ng):
                            g = c0 + gi
                            sl = slice(gi * FA, (gi + 1) * FA)
                            nc.tensor.matmul(
                                out=pv[:, sl],
                                lhsT=ST_t[:, g * 128:(g + 1) * 128],
                                rhs=xr_b, start=True, stop=False)
                            nc.tensor.matmul(
                                out=pv[:, sl], lhsT=ident_sb[:],
                                rhs=xlg[:, g, 0:FA], start=False, stop=True)
                        r_t = r_pool.tile([128, CH * FA], BF, tag="r")
                        nc.scalar.activation(r_t[:, 0:ng * FA],
                                             pv[:, 0:ng * FA], AF.Relu,
                                             scale=1.0 - NEG_SLOPE)
                        if not USE_TTR:
                            # strided psum read on ACT (DVE is the bottleneck)
                            nc.scalar.activation(
                                av_t[:, c0:c0 + ng],
                                pv[:, FA - 1:ng * FA:FA], AF.Copy)
                        for gi in range(ng):
                            g = c0 + gi
                            scr = scr_pool.tile([128, F], BF, tag="scr")
                            if USE_TTR:
                                nc.vector.tensor_tensor_reduce(
                                    out=scr[:],
                                    in0=r_t[:, gi * FA:gi * FA + F],
                                    in1=att_sb[:, 0:F],
                                    scale=1.0,
                                    scalar=pv[:, gi * FA + F:(gi + 1) * FA],
                                    op0=ALU.mult, op1=ALU.add,
                                    accum_out=logit_t[:, g:g + 1])
                            else:
                                # att.r into logit, then add the 0.2*att.v
                                # column on ACT (reads psum col F)
                                nc.vector.scalar_tensor_tensor(
                                    out=scr[:],
                                    in0=r_t[:, gi * FA:gi * FA + F],
                                    scalar=1.0, in1=att_sb[:, 0:F],
                                    op0=ALU.mult, op1=ALU.mult,
                                    accum_out=lacc_t[:, g:g + 1])

                    if not USE_TTR:
                        nc.vector.tensor_tensor(
                            out=logit_t[:], in0=lacc_t[:], in1=av_t[:],
                            op=ALU.add)
                    if DEBUG and dbg:
                        nc.sync.dma_start(out=logit1_dbg[:, gs:gs + G],
                                          in_=logit_t[:])
                    w_t = sm_pool.tile([128, G], F32, tag="w")
                    nc.scalar.activation(w_t[:], logit_t[:], AF.Exp)

                    # one matmul per group: rhs cols [xl(F) | a_l | 1.0]
                    # -> psum [U(F) | junk | s]
                    pU = ps_u.tile([BLK, F + 2], F32, tag="pU")
                    for g in range(G):
                        sw = sw_pool.tile([128, BLK], BF, tag="sw")
                        nc.vector.tensor_scalar(
                            out=sw[:], in0=iota_sb[:],
                            scalar1=dst_t[:, g:g + 1],
                            scalar2=w_t[:, g:g + 1],
                            op0=ALU.is_equal, op1=ALU.mult)
                        nc.tensor.matmul(out=pU[:], lhsT=sw[:, 0:BLK],
                                         rhs=xlg[:, g, 0:F + 2],
                                         start=(g == 0), stop=(g == G - 1))

                    if DEBUG and dbg:
                        ud = np_pool.tile([BLK, F + 1], F32, tag="ud")
                        nc.vector.tensor_scalar_add(ud[:, 0:F], pU[:, 0:F], 0.0)
                        nc.vector.tensor_scalar_add(
                            ud[:, F:F + 1], pU[:, F + 1:F + 2], 0.0)
                        nc.sync.dma_start(
                            out=us_dbg[b * BLK:(b + 1) * BLK, 0:F + 1],
                            in_=ud[:])
                    rs = sm_pool.tile([BLK, 1], F32, tag="rs")
                    nc.vector.reciprocal(rs[:], pU[:, F + 1:F + 2])
                    y = np_pool.tile([BLK, F], F32, tag="y")
                    nc.vector.scalar_tensor_tensor(
                        out=y[:], in0=pU[:, 0:F], scalar=rs[:],
                        in1=b_sb[0:BLK, 0:F],
                        op0=ALU.mult, op1=ALU.add)
                    mn = np_pool.tile([BLK, F], F32, tag="mn")
                    nc.vector.tensor_scalar_min(mn[:], y[:], 0.0)
                    exm = np_pool.tile([BLK, F], F32, tag="exm")
                    nc.scalar.activation(exm[:], mn[:], AF.Exp)
                    sink(b, y, exm)

            # ================= layer-1 edge phase =================
            def sink1(b, y, exm):
                # h = elu(y)+1 = max(y,0) + exp(min(y,0)); -1 folded into L2
                h = np_pool.tile([BLK, H], BF, tag="h")
                nc.vector.scalar_tensor_tensor(
                    out=h[:], in0=y[:], scalar=0.0, in1=exm[:],
                    op0=ALU.max, op1=ALU.add)
                pT = ps_t.tile([H, BLK], F32, tag="pT")
                nc.tensor.matmul(out=pT[:], lhsT=h[:],
                                 rhs=ident_sb[0:BLK, 0:BLK],
                                 start=True, stop=True)
                nc.scalar.activation(hT_sb[:, b * BLK:(b + 1) * BLK],
                                     pT[:], AF.Copy)
                if DEBUG:
                    nc.sync.dma_start(out=h_dbg[b * BLK:(b + 1) * BLK, :],
                                      in_=h[:])

            edge_layer(H, FA1, T1, CH1, xl1_d, xr1_sb, att1_sb, b1_sb, sink1,
                       dbg=DEBUG_LAYER == 1)

            # ================= layer-2 matmul phase =================
            for j in range(NB):
                lhsT = hT_sb[:, j * BLK:(j + 1) * BLK]
                pa = ps_mm.tile([BLK, FA2], F32, tag="mm")
                nc.tensor.matmul(out=pa[:], lhsT=lhsT, rhs=Wl2_sb[:],
                                 start=True, stop=True)
                agt = mm_pool.tile([BLK, FA2], BF, tag="agt2")
                nc.scalar.activation(agt[:], pa[:], AF.Copy)
                nc.sync.dma_start(out=ag2_d[j * BLK:(j + 1) * BLK, 0:FA2],
                                  in_=agt[:])
                pb = ps_mm.tile([BLK, FA2], F32, tag="mm")
                nc.tensor.matmul(out=pb[:], lhsT=lhsT, rhs=Wr2_sb[:],
                                 start=True, stop=True)
                nc.scalar.activation(xr2_sb[0:BLK, j * FA2:(j + 1) * FA2],
                                     pb[:], AF.Copy)

            tc.strict_bb_all_engine_barrier()
            if NC_N == 1:
                nc.sync.dma_start(out=xl2_d[:], in_=ag2_d[:])
            else:
                nc.gpsimd.collective_compute(
                    "AllGather", ALU.bypass,
                    replica_groups=[list(range(NC_N))],
                    ins=[ag2_d[:]], outs=[xl2_d[:]])
            tc.strict_bb_all_engine_barrier()

            # ================= layer-2 edge phase =================
            def sink2(b, y, exm):
                # out = elu(y) = (y - min(y,0)) + (exp(min(y,0)) - 1)
                mn2 = np_pool.tile([BLK, OUT], F32, tag="ymn")
                nc.vector.scalar_tensor_tensor(
                    out=mn2[:], in0=y[:], scalar=0.0, in1=exm[:],
                    op0=ALU.max, op1=ALU.add)
                h2 = np_pool.tile([BLK, OUT], F32, tag="h2")
                nc.vector.tensor_scalar_add(h2[:], mn2[:], -1.0)
                nc.sync.dma_start(out=out_d[b * BLK:(b + 1) * BLK, :],
                                  in_=h2[:])

            edge_layer(OUT, FA2, T2, CH2, xl2_d, xr2_sb, att2_sb, b2_sb, sink2,
                       dbg=DEBUG_LAYER == 2)

    nc.compile()
    return nc


_CACHE = {}


def kernel(x, edge_index, edge_attr, Wl1, Wr1, We1, att1, b1,
           Wl2, Wr2, We2, att2, b2, _trace=False):
    from concourse.bass_utils import run_bass_kernel_spmd

    x = np.asarray(x, np.float32)
    edge_index = np.asarray(edge_index)
    edge_attr = np.asarray(edge_attr, np.float32)

    core_inputs, g_lo, g_hi, g_start, g_total, meta = prep_inputs(
        x, edge_index, edge_attr)

    if meta not in _CACHE:
        _CACHE[meta] = build_program(g_lo, g_hi, g_start, g_total)
    nc = _CACHE[meta]

    consts = build_consts(Wl1, Wr1, We1, att1, b1, Wl2, Wr2, We2, att2, b2)
    in_maps = []
    for c in range(NC_N):
        m = dict(consts)
        m.update(core_inputs[c])
        in_maps.append(m)

    res = run_bass_kernel_spmd(nc, in_maps, list(range(NC_N)), trace=_trace)
    LAST_EXEC_NS[0] = res.exec_time_ns
    LAST_RESULTS[0] = res.results
    out = np.concatenate([res.results[c]["out_slice"] for c in range(NC_N)],
                         axis=0)
    return out.astype(np.float32)


# revision 45
# speedup vs baseline: 1.0932x; 1.0932x over previous
"""Two-layer GATv2 (heads=1) on 8 Trainium2 NeuronCores.

Sharding: nodes partitioned by dst across 8 cores (6250 each), dst-blocks of
125 nodes. Edges of a block are sorted lo/hi by src (split at 32768 so
indices fit int16) and processed in groups of 128.

Per layer:
  - matmul phase: xl = x@[Wl | 0.2*Wl@att], xr likewise (PE, bf16); xl rows
    AllGathered into a [N, T] row table (T = 256/128 cols, 512/256B rows).
  - edge phase per dst-block:
      xlg  <- batched dma_gather of xl rows (two gathers: lo/hi table halves)
      psum <- S_T_aug.T @ xr_aug  (one-hot dst select + We*ea + const row)
            + I.T @ xlg           (adds xl[src]; col F accumulates 0.2*att.v)
      r    <- Relu(0.8*psum) on ACT      (lrelu(v) = 0.2v + relu(0.8v))
      logit<- ttr(r, att, mult, add, init=psum_col_F)  -> 0.2*att.v + att.r
      w    <- Exp(logit)
      sw   <- (iota==dst)*w (bf16 one-hot), U += sw.T@xlg, s += sw.T@1 (PE)
      h    <- elu(U/s + b) (+1 folded into next layer's const row)
Weights replicated; layer-2 constant correction (elu's -1) folded into ST
row 126 x xr row 126.
"""
import sys

sys.path.insert(0, '/opt/trn_rl_repo')

import numpy as np
import ml_dtypes
from contextlib import ExitStack

import concourse.bass as bass
import concourse.tile as tile
import concourse.bacc as bacc
from concourse import mybir

BF16 = ml_dtypes.bfloat16
F32 = mybir.dt.float32
BF = mybir.dt.bfloat16
I16 = mybir.dt.int16
AF = mybir.ActivationFunctionType
ALU = mybir.AluOpType

NEG_SLOPE = 0.2

# Problem geometry (nn_Affinity_GAT_75557064671579)
N = 50000
E = 800000
F_IN = 128
H = 128
OUT = 64
NC_N = 8
NLOC = N // NC_N        # 6250 nodes per core
BLK = 125               # dst nodes per block
NB = NLOC // BLK        # 50 blocks per core
LO = 32768              # src table split (int16 index limit)
PAD_DST = 1000.0

LAST_EXEC_NS = [None]
LAST_RESULTS = [None]
DEBUG = False
DEBUG_LAYER = 1
NQUEUES = 4     # SWDGE queues used for gathers
MAXG = 8        # groups per gather instruction (1024 idxs; ring cap ~65/dma)
USE_TTR = False  # InstTensorTensorReduce wedges real HW; use stt + av add


def prep_inputs(x, edge_index, edge_attr):
    """Host-side graph prep: self loops, dst-shard, block layout, lo/hi
    src split, gather indices, one-hot S_T_aug."""
    src = edge_index[0].astype(np.int64)
    dst = edge_index[1].astype(np.int64)
    ea = edge_attr[:, 0].astype(np.float32)

    # add_self_loops with fill_value='mean' over incoming edges
    cnt = np.bincount(dst, minlength=N).astype(np.float32)
    asum = np.bincount(dst, weights=ea, minlength=N).astype(np.float32)
    loop_attr = (asum / np.maximum(cnt, 1.0)).astype(np.float32)
    src_all = np.concatenate([src, np.arange(N, dtype=np.int64)])
    dst_all = np.concatenate([dst, np.arange(N, dtype=np.int64)])
    ea_all = np.concatenate([ea, loop_attr])

    core_of = dst_all // NLOC
    percore = []
    nlo = np.zeros((NC_N, NB), np.int64)
    nhi = np.zeros((NC_N, NB), np.int64)
    for c in range(NC_N):
        m = core_of == c
        s_c = src_all[m]
        d_c = dst_all[m] - c * NLOC
        a_c = ea_all[m]
        b_c = d_c // BLK
        islo = s_c < LO
        key = b_c * 2 + (~islo).astype(np.int64)
        order = np.argsort(key, kind='stable')
        s_c, d_c, a_c, b_c, islo = (s_c[order], d_c[order], a_c[order],
                                    b_c[order], islo[order])
        percore.append((s_c, d_c, a_c, b_c, islo))
        nlo[c] = np.bincount(b_c[islo], minlength=NB)
        nhi[c] = np.bincount(b_c[~islo], minlength=NB)

    g_lo = ((nlo.max(axis=0) + 127) // 128).astype(np.int64)
    g_hi = ((nhi.max(axis=0) + 127) // 128).astype(np.int64)
    g_all = g_lo + g_hi
    g_start = np.concatenate([[0], np.cumsum(g_all)])
    g_total = int(g_start[-1])

    core_inputs = []
    for c in range(NC_N):
        s_c, d_c, a_c, b_c, islo = percore[c]
        idx16 = np.zeros((128, 8 * g_total), np.int16)
        dstc = np.full((128, g_total), PAD_DST, np.float32)
        ST = np.zeros((128, 128 * g_total), np.float32)

        # slot index per edge: block base + lo slot / hi region slot
        off_l = np.zeros(NB, np.int64)
        off_h = np.zeros(NB, np.int64)
        slot = np.zeros(len(s_c), np.int64)
        # edges are sorted by (block, hi) so within each (block, half)
        # they are consecutive
        for b in range(NB):
            lo_n = int(nlo[c, b])
            hi_n = int(nhi[c, b])
            base = g_start[b] * 128
            sel_lo = (b_c == b) & islo
            sel_hi = (b_c == b) & ~islo
            slot[sel_lo] = base + np.arange(lo_n)
            slot[sel_hi] = base + 128 * g_lo[b] + np.arange(hi_n)

        din = (d_c % BLK).astype(np.int64)
        ST[din, slot] = 1.0
        ST[126, slot] = 1.0
        ST[127, slot] = a_c
        dstc[slot % 128, slot // 128] = din.astype(np.float32)
        # Q7 cpu pair for queue q reads idxs from partitions [32q, 32q+32):
        # replicate the [16, cols] block across all 128 partitions.
        idxval = np.where(islo, s_c, s_c - LO).astype(np.int16)
        idx16[slot % 16, slot // 16] = idxval
        idx16 = np.tile(idx16[0:16], (8, 1))

        xT = np.ascontiguousarray(
            x[c * NLOC:(c + 1) * NLOC].T).astype(BF16)  # [F_IN, NLOC]
        core_inputs.append(dict(
            xT_slice=xT,
            idx_d=idx16,
            dstc_d=dstc,
            ST_d=ST.astype(BF16),
        ))
    meta = (tuple(g_lo.tolist()), tuple(g_hi.tolist()))
    return core_inputs, g_lo, g_hi, g_start, g_total, meta


def build_consts(Wl1, Wr1, We1, att1, b1, Wl2, Wr2, We2, att2, b2):
    att1 = np.asarray(att1, np.float32)
    att2 = np.asarray(att2, np.float32)
    Wl1 = np.asarray(Wl1, np.float32)
    Wr1 = np.asarray(Wr1, np.float32)
    Wl2 = np.asarray(Wl2, np.float32)
    Wr2 = np.asarray(Wr2, np.float32)
    We1 = np.asarray(We1, np.float32).reshape(-1)   # [H]
    We2 = np.asarray(We2, np.float32).reshape(-1)   # [OUT]

    def aug(W, att):
        return np.concatenate([W, NEG_SLOPE * (W @ att)[:, None]],
                              axis=1).astype(BF16)

    c2 = -(Wl2.sum(axis=0) + Wr2.sum(axis=0))       # [OUT]
    we1row = np.concatenate([We1, [NEG_SLOPE * float(We1 @ att1)]])
    we2row = np.concatenate([We2, [NEG_SLOPE * float(We2 @ att2)]])
    c2row = np.concatenate([c2, [NEG_SLOPE * float(c2 @ att2)]])

    return dict(
        Wl1aug=aug(Wl1, att1), Wr1aug=aug(Wr1, att1),
        Wl2aug=aug(Wl2, att2), Wr2aug=aug(Wr2, att2),
        We1rep=np.tile(we1row[None, :], (1, NB)).astype(BF16),
        We2rep=np.tile(we2row[None, :], (1, NB)).astype(BF16),
        Cr2rep=np.tile(c2row[None, :], (1, NB)).astype(BF16),
        att1t=np.tile(att1[None, :], (128, 1)).astype(BF16),
        att2t=np.tile(att2[None, :], (128, 1)).astype(BF16),
        b1t=np.tile(np.asarray(b1, np.float32)[None, :], (128, 1)),
        # U2 accumulates raw h@Wl2 rows (each +colsum(Wl2) vs (h-1)@Wl2);
        # U/s is a convex combination, so subtract it once via the bias.
        b2t=np.tile((np.asarray(b2, np.float32) - Wl2.sum(axis=0))[None, :],
                    (128, 1)),
        iota_row=np.tile(np.arange(BLK, dtype=np.float32)[None, :],
                         (128, 1)).astype(BF16),
        ident=np.eye(128, dtype=np.float32).astype(BF16),
        ones_col=np.ones((128, 1), np.float32).astype(BF16),
    )


def build_program(g_lo, g_hi, g_start, g_total):
    nc = bacc.Bacc("TRN2", target_bir_lowering=False, debug=False,
                   num_devices=NC_N, num_swdge_queues=NQUEUES)

    T1, T2 = 256, 128           # gather table row widths (cols)
    FA1, FA2 = H + 1, OUT + 1   # v columns incl. att.v accumulator
    CH1, CH2 = 3, 7             # psum-bank groups per chunk (<=512 f32)

    # ---- DRAM I/O ----
    xT_d = nc.dram_tensor("xT_slice", [F_IN, NLOC], BF, kind="ExternalInput")
    idx_d = nc.dram_tensor("idx_d", [128, 8 * g_total], I16, kind="ExternalInput")
    dstc_d = nc.dram_tensor("dstc_d", [128, g_total], F32, kind="ExternalInput")
    ST_d = nc.dram_tensor("ST_d", [128, 128 * g_total], BF, kind="ExternalInput")
    Wl1_d = nc.dram_tensor("Wl1aug", [F_IN, FA1], BF, kind="ExternalInput")
    Wr1_d = nc.dram_tensor("Wr1aug", [F_IN, FA1], BF, kind="ExternalInput")
    Wl2_d = nc.dram_tensor("Wl2aug", [H, FA2], BF, kind="ExternalInput")
    Wr2_d = nc.dram_tensor("Wr2aug", [H, FA2], BF, kind="ExternalInput")
    We1_d = nc.dram_tensor("We1rep", [1, NB * FA1], BF, kind="ExternalInput")
    We2_d = nc.dram_tensor("We2rep", [1, NB * FA2], BF, kind="ExternalInput")
    Cr2_d = nc.dram_tensor("Cr2rep", [1, NB * FA2], BF, kind="ExternalInput")
    att1_d = nc.dram_tensor("att1t", [128, H], BF, kind="ExternalInput")
    att2_d = nc.dram_tensor("att2t", [128, OUT], BF, kind="ExternalInput")
    b1_d = nc.dram_tensor("b1t", [128, H], F32, kind="ExternalInput")
    b2_d = nc.dram_tensor("b2t", [128, OUT], F32, kind="ExternalInput")
    iota_d = nc.dram_tensor("iota_row", [128, BLK], BF, kind="ExternalInput")
    ident_d = nc.dram_tensor("ident", [128, 128], BF, kind="ExternalInput")
    ones_d = nc.dram_tensor("ones_col", [128, 1], BF, kind="ExternalInput")
    out_d = nc.dram_tensor("out_slice", [NLOC, OUT], F32, kind="ExternalOutput")

    if DEBUG:
        logit1_dbg = nc.dram_tensor("logit1_dbg", [128, g_total], F32,
                                    kind="ExternalOutput")
        h_dbg = nc.dram_tensor("h_dbg", [NLOC, H], BF, kind="ExternalOutput")
        us_dbg = nc.dram_tensor("us_dbg", [NLOC, H + 1], F32,
                                kind="ExternalOutput")

    ag1_d = nc.dram_tensor("ag1_in", [NLOC, T1], BF)
    xl1_d = nc.dram_tensor("xl1_full", [N, T1], BF, addr_space="Shared")
    ag2_d = nc.dram_tensor("ag2_in", [NLOC, T2], BF)
    xl2_d = nc.dram_tensor("xl2_full", [N, T2], BF, addr_space="Shared")

    qn = [0]

    with tile.TileContext(nc) as tc:
        with ExitStack() as ctx:
            cpool = ctx.enter_context(tc.tile_pool(name="const", bufs=1))
            mm_pool = ctx.enter_context(tc.tile_pool(name="mmx", bufs=3))
            st_pool = ctx.enter_context(tc.tile_pool(name="stp", bufs=3))
            xlg_pool = ctx.enter_context(tc.tile_pool(name="xlg", bufs=3))
            idx_pool = ctx.enter_context(tc.tile_pool(name="idxp", bufs=3))
            sm_pool = ctx.enter_context(tc.tile_pool(name="small", bufs=3))
            r_pool = ctx.enter_context(tc.tile_pool(name="rz", bufs=3))
            scr_pool = ctx.enter_context(tc.tile_pool(name="scr", bufs=4))
            sw_pool = ctx.enter_context(tc.tile_pool(name="sw", bufs=4))
            np_pool = ctx.enter_context(tc.tile_pool(name="nodep", bufs=3))
            ps_mm = ctx.enter_context(tc.tile_pool(name="psmm", bufs=1, space="PSUM"))
            ps_v = ctx.enter_context(tc.tile_pool(name="psv", bufs=2, space="PSUM"))
            ps_u = ctx.enter_context(tc.tile_pool(name="psu", bufs=2, space="PSUM"))
            ps_t = ctx.enter_context(tc.tile_pool(name="pst", bufs=1, space="PSUM"))

            def cload(name, dram, shape, dt=BF):
                t = cpool.tile(shape, dt, tag=name)
                nc.sync.dma_start(out=t[:], in_=dram[:])
                return t

            xT_sb = cload("xT", xT_d, [F_IN, NLOC])
            Wl1_sb = cload("Wl1", Wl1_d, [F_IN, FA1])
            Wr1_sb = cload("Wr1", Wr1_d, [F_IN, FA1])
            Wl2_sb = cload("Wl2", Wl2_d, [H, FA2])
            Wr2_sb = cload("Wr2", Wr2_d, [H, FA2])
            att1_sb = cload("att1", att1_d, [128, H])
            att2_sb = cload("att2", att2_d, [128, OUT])
            b1_sb = cload("b1", b1_d, [128, H], F32)
            b2_sb = cload("b2", b2_d, [128, OUT], F32)
            iota_sb = cload("iota", iota_d, [128, BLK])
            ident_sb = cload("ident", ident_d, [128, 128])
            ones_sb = cload("ones", ones_d, [128, 1])

            hT_sb = cpool.tile([H, NLOC], BF, tag="hT")
            xr1_sb = cpool.tile([128, NB * FA1], BF, tag="xr1")
            xr2_sb = cpool.tile([128, NB * FA2], BF, tag="xr2")
            nc.vector.memset(xr1_sb[96:128, :], 0.0)
            nc.vector.memset(xr2_sb[96:128, :], 0.0)
            nc.sync.dma_start(out=xr1_sb[127:128, :], in_=We1_d[:])
            nc.sync.dma_start(out=xr2_sb[126:127, :], in_=Cr2_d[:])
            nc.sync.dma_start(out=xr2_sb[127:128, :], in_=We2_d[:])

            # tail columns of the gather-table rows: col FA (right after
            # the a_l column) is a constant 1.0 so one U-matmul also
            # accumulates s = sum(w) in psum col F+1; the rest are zero.
            zt = cpool.tile([BLK, T1 - FA1], BF, tag="zt")
            nc.vector.memset(zt[:], 0.0)
            nc.vector.memset(zt[:, 0:1], 1.0)
            for j in range(NB):
                rows = slice(j * BLK, (j + 1) * BLK)
                nc.sync.dma_start(out=ag1_d[rows, FA1:T1],
                                  in_=zt[:, 0:T1 - FA1])
                nc.sync.dma_start(out=ag2_d[rows, FA2:T2],
                                  in_=zt[:, 0:T2 - FA2])

            # ================= layer-1 matmul phase =================
            for j in range(NB):
                lhsT = xT_sb[:, j * BLK:(j + 1) * BLK]
                pa = ps_mm.tile([BLK, FA1], F32, tag="mm")
                nc.tensor.matmul(out=pa[:], lhsT=lhsT, rhs=Wl1_sb[:],
                                 start=True, stop=True)
                agt = mm_pool.tile([BLK, FA1], BF, tag="agt")
                nc.scalar.activation(agt[:], pa[:], AF.Copy)
                nc.sync.dma_start(out=ag1_d[j * BLK:(j + 1) * BLK, 0:FA1],
                                  in_=agt[:])
                pb = ps_mm.tile([BLK, FA1], F32, tag="mm")
                nc.tensor.matmul(out=pb[:], lhsT=lhsT, rhs=Wr1_sb[:],
                                 start=True, stop=True)
                nc.scalar.activation(xr1_sb[0:BLK, j * FA1:(j + 1) * FA1],
                                     pb[:], AF.Copy)

            tc.strict_bb_all_engine_barrier()
            if NC_N == 1:
                nc.sync.dma_start(out=xl1_d[:], in_=ag1_d[:])
            else:
                nc.gpsimd.collective_compute(
                    "AllGather", ALU.bypass,
                    replica_groups=[list(range(NC_N))],
                    ins=[ag1_d[:]], outs=[xl1_d[:]])
            tc.strict_bb_all_engine_barrier()

            def edge_layer(F, FA, T, CH, xl_full, xr_sb, att_sb, b_sb, sink,
                           dbg=False):
                for b in range(NB):
                    GL, GH = int(g_lo[b]), int(g_hi[b])
                    G = GL + GH
                    gs = int(g_start[b])
                    ST_t = st_pool.tile([128, 128 * G], BF, tag="st")
                    nc.sync.dma_start(
                        out=ST_t[:],
                        in_=ST_d[:, 128 * gs:128 * (gs + G)])
                    idx_t = idx_pool.tile([128, 8 * G], I16, tag="idx")
                    nc.sync.dma_start(
                        out=idx_t[:], in_=idx_d[:, 8 * gs:8 * (gs + G)])
                    dst_t = sm_pool.tile([128, G], F32, tag="dst")
                    nc.sync.dma_start(
                        out=dst_t[:], in_=dstc_d[:, gs:gs + G])

                    xlg = xlg_pool.tile([128, G, T], BF, tag="xlg")
                    for base, gn, tab in ((0, GL, xl_full[0:LO, :]),
                                          (GL, GH, xl_full[LO:N, :])):
                        for q0 in range(0, gn, MAXG):
                            n = min(MAXG, gn - q0)
                            o = base + q0
                            nc.gpsimd.dma_gather(
                                out_ap=xlg[:, o:o + n, :], in_ap=tab,
                                idxs_ap=idx_t[:, 8 * o:8 * (o + n)],
                                num_idxs=128 * n, num_idxs_reg=128 * n,
                                elem_size=T, queue_num=qn[0] % NQUEUES)
                            qn[0] += 1

                    logit_t = sm_pool.tile([128, G], F32, tag="logit")
                    if not USE_TTR:
                        lacc_t = sm_pool.tile([128, G], F32, tag="lacc")
                        av_t = sm_pool.tile([128, G], F32, tag="av")
                    xr_b = xr_sb[:, b * FA:(b + 1) * FA]
                    for c0 in range(0, G, CH):
                        ng = min(CH, G - c0)
                        pv = ps_v.tile([128, CH * FA], F32, tag="pv")
                        for gi in range(ng):
                            g = c0 + gi
                            sl = slice(gi * FA, (gi + 1) * FA)
                            nc.tensor.matmul(
                                out=pv[:, sl],
                                lhsT=ST_t[:, g * 128:(g + 1) * 128],
                                rhs=xr_b, start=True, stop=False)
                            nc.tensor.matmul(
                                out=pv[:, sl], lhsT=ident_sb[:],
                                rhs=xlg[:, g, 0:FA], start=False, stop=True)
                        r_t = r_pool.tile([128, CH * FA], BF, tag="r")
                        nc.scalar.activation(r_t[:, 0:ng * FA],
                                             pv[:, 0:ng * FA], AF.Relu,
                                             scale=1.0 - NEG_SLOPE)
                        if not USE_TTR:
                            nc.vector.tensor_scalar_add(
                                av_t[:, c0:c0 + ng],
                                pv[:, FA - 1:ng * FA:FA], 0.0)
                        for gi in range(ng):
                            g = c0 + gi
                            scr = scr_pool.tile([128, F], BF, tag="scr")
                            if USE_TTR:
                                nc.vector.tensor_tensor_reduce(
                                    out=scr[:],
                                    in0=r_t[:, gi * FA:gi * FA + F],
                                    in1=att_sb[:, 0:F],
                                    scale=1.0,
                                    scalar=pv[:, gi * FA + F:(gi + 1) * FA],
                                    op0=ALU.mult, op1=ALU.add,
                                    accum_out=logit_t[:, g:g + 1])
                            else:
                                # att.r into logit, then add the 0.2*att.v
                                # column on ACT (reads psum col F)
                                nc.vector.scalar_tensor_tensor(
                                    out=scr[:],
                                    in0=r_t[:, gi * FA:gi * FA + F],
                                    scalar=1.0, in1=att_sb[:, 0:F],
                                    op0=ALU.mult, op1=ALU.mult,
                                    accum_out=lacc_t[:, g:g + 1])

                    if not USE_TTR:
                        nc.vector.tensor_tensor(
                            out=logit_t[:], in0=lacc_t[:], in1=av_t[:],
                            op=ALU.add)
                    if DEBUG and dbg:
                        nc.sync.dma_start(out=logit1_dbg[:, gs:gs + G],
                                          in_=logit_t[:])
                    w_t = sm_pool.tile([128, G], F32, tag="w")
                    nc.scalar.activation(w_t[:], logit_t[:], AF.Exp)

                    # one matmul per group: rhs cols [xl(F) | a_l | 1.0]
                    # -> psum [U(F) | junk | s]
                    pU = ps_u.tile([BLK, F + 2], F32, tag="pU")
                    for g in range(G):
                        sw = sw_pool.tile([128, BLK], BF, tag="sw")
                        nc.vector.tensor_scalar(
                            out=sw[:], in0=iota_sb[:],
                            scalar1=dst_t[:, g:g + 1],
                            scalar2=w_t[:, g:g + 1],
                            op0=ALU.is_equal, op1=ALU.mult)
                        nc.tensor.matmul(out=pU[:], lhsT=sw[:, 0:BLK],
                                         rhs=xlg[:, g, 0:F + 2],
                                         start=(g == 0), stop=(g == G - 1))

                    if DEBUG and dbg:
                        ud = np_pool.tile([BLK, F + 1], F32, tag="ud")
                        nc.vector.tensor_scalar_add(ud[:, 0:F], pU[:, 0:F], 0.0)
                        nc.vector.tensor_scalar_add(
                            ud[:, F:F + 1], pU[:, F + 1:F + 2], 0.0)
                        nc.sync.dma_start(
                            out=us_dbg[b * BLK:(b + 1) * BLK, 0:F + 1],
                            in_=ud[:])
                    rs = sm_pool.tile([BLK, 1], F32, tag="rs")
                    nc.vector.reciprocal(rs[:], pU[:, F + 1:F + 2])
                    y = np_pool.tile([BLK, F], F32, tag="y")
                    nc.vector.scalar_tensor_tensor(
                        out=y[:], in0=pU[:, 0:F], scalar=rs[:],
                        in1=b_sb[0:BLK, 0:F],
                        op0=ALU.mult, op1=ALU.add)
                    mn = np_pool.tile([BLK, F], F32, tag="mn")
                    nc.vector.tensor_scalar_min(mn[:], y[:], 0.0)
                    exm = np_pool.tile([BLK, F], F32, tag="exm")
                    nc.scalar.activation(exm[:], mn[:], AF.Exp)
                    sink(b, y, exm)

            # ================= layer-1 edge phase =================
            def sink1(b, y, exm):
                # h = elu(y)+1 = max(y,0) + exp(min(y,0)); -1 folded into L2
                h = np_pool.tile([BLK, H], BF, tag="h")
                nc.vector.scalar_tensor_tensor(
                    out=h[:], in0=y[:], scalar=0.0, in1=exm[:],
                    op0=ALU.max, op1=ALU.add)
                pT = ps_t.tile([H, BLK], F32, tag="pT")
                nc.tensor.matmul(out=pT[:], lhsT=h[:],
                                 rhs=ident_sb[0:BLK, 0:BLK],
                                 start=True, stop=True)
                nc.scalar.activation(hT_sb[:, b * BLK:(b + 1) * BLK],
                                     pT[:], AF.Copy)
                if DEBUG:
                    nc.sync.dma_start(out=h_dbg[b * BLK:(b + 1) * BLK, :],
                                      in_=h[:])

            edge_layer(H, FA1, T1, CH1, xl1_d, xr1_sb, att1_sb, b1_sb, sink1,
                       dbg=DEBUG_LAYER == 1)

            # ================= layer-2 matmul phase =================
            for j in range(NB):
                lhsT = hT_sb[:, j * BLK:(j + 1) * BLK]
                pa = ps_mm.tile([BLK, FA2], F32, tag="mm")
                nc.tensor.matmul(out=pa[:], lhsT=lhsT, rhs=Wl2_sb[:],
                                 start=True, stop=True)
                agt = mm_pool.tile([BLK, FA2], BF, tag="agt2")
                nc.scalar.activation(agt[:], pa[:], AF.Copy)
                nc.sync.dma_start(out=ag2_d[j * BLK:(j + 1) * BLK, 0:FA2],
                                  in_=agt[:])
                pb = ps_mm.tile([BLK, FA2], F32, tag="mm")
                nc.tensor.matmul(out=pb[:], lhsT=lhsT, rhs=Wr2_sb[:],
                                 start=True, stop=True)
                nc.scalar.activation(xr2_sb[0:BLK, j * FA2:(j + 1) * FA2],
                                     pb[:], AF.Copy)

            tc.strict_bb_all_engine_barrier()
            if NC_N == 1:
                nc.sync.dma_start(out=xl2_d[:], in_=ag2_d[:])
            else:
                nc.gpsimd.collective_compute(
                    "AllGather", ALU.bypass,
                    replica_groups=[list(range(NC_N))],
                    ins=[ag2_d[:]], outs=[xl2_d[:]])
            tc.strict_bb_all_engine_barrier()

            # ================= layer-2 edge phase =================
            def sink2(b, y, exm):
                # out = elu(y) = (y - min(y,0)) + (exp(min(y,0)) - 1)
                mn2 = np_pool.tile([BLK, OUT], F32, tag="ymn")
                nc.vector.scalar_tensor_tensor(
                    out=mn2[:], in0=y[:], scalar=0.0, in1=exm[:],
                    op0=ALU.max, op1=ALU.add)
                h2 = np_pool.tile([BLK, OUT], F32, tag="h2")
                nc.vector.tensor_scalar_add(h2[:], mn2[:], -1.0)
                nc.sync.dma_start(out=out_d[b * BLK:(b + 1) * BLK, :],
                                  in_=h2[:])

            edge_layer(OUT, FA2, T2, CH2, xl2_d, xr2_sb, att2_sb, b2_sb, sink2,
                       dbg=DEBUG_LAYER == 2)

    nc.compile()
    return nc


_CACHE = {}


def kernel(x, edge_index, edge_attr, Wl1, Wr1, We1, att1, b1,
           Wl2, Wr2, We2, att2, b2, _trace=False):
    from concourse.bass_utils import run_bass_kernel_spmd

    x = np.asarray(x, np.float32)
    edge_index = np.asarray(edge_index)
    edge_attr = np.asarray(edge_attr, np.float32)

    core_inputs, g_lo, g_hi, g_start, g_total, meta = prep_inputs(
        x, edge_index, edge_attr)

    if meta not in _CACHE:
        _CACHE[meta] = build_program(g_lo, g_hi, g_start, g_total)
    nc = _CACHE[meta]

    consts = build_consts(Wl1, Wr1, We1, att1, b1, Wl2, Wr2, We2, att2, b2)
    in_maps = []
    for c in range(NC_N):
        m = dict(consts)
        m.update(core_inputs[c])
        in_maps.append(m)

    res = run_bass_kernel_spmd(nc, in_maps, list(range(NC_N)), trace=_trace)
    LAST_EXEC_NS[0] = res.exec_time_ns
    LAST_RESULTS[0] = res.results
    out = np.concatenate([res.results[c]["out_slice"] for c in range(NC_N)],
                         axis=0)
    return out.astype(np.float32)


# revision 48
# speedup vs baseline: 1.1430x; 1.0455x over previous
"""Two-layer GATv2 (heads=1) on 8 Trainium2 NeuronCores.

Sharding: nodes partitioned by dst across 8 cores (6250 each), dst-blocks of
125 nodes. Edges of a block are sorted lo/hi by src (split at 32768 so
indices fit int16) and processed in groups of 128.

Per layer:
  - matmul phase: xl = x@[Wl | 0.2*Wl@att], xr likewise (PE, bf16); xl rows
    AllGathered into a [N, T] row table (T = 256/128 cols, 512/256B rows).
  - edge phase per dst-block:
      xlg  <- batched dma_gather of xl rows (two gathers: lo/hi table halves)
      psum <- S_T_aug.T @ xr_aug  (one-hot dst select + We*ea + const row)
            + I.T @ xlg           (adds xl[src]; col F accumulates 0.2*att.v)
      r    <- Relu(0.8*psum) on ACT      (lrelu(v) = 0.2v + relu(0.8v))
      logit<- ttr(r, att, mult, add, init=psum_col_F)  -> 0.2*att.v + att.r
      w    <- Exp(logit)
      sw   <- (iota==dst)*w (bf16 one-hot), U += sw.T@xlg, s += sw.T@1 (PE)
      h    <- elu(U/s + b) (+1 folded into next layer's const row)
Weights replicated; layer-2 constant correction (elu's -1) folded into ST
row 126 x xr row 126.
"""
import sys

sys.path.insert(0, '/opt/trn_rl_repo')

import numpy as np
import ml_dtypes
from contextlib import ExitStack

import concourse.bass as bass
import concourse.tile as tile
import concourse.bacc as bacc
from concourse import mybir

BF16 = ml_dtypes.bfloat16
F32 = mybir.dt.float32
BF = mybir.dt.bfloat16
I16 = mybir.dt.int16
AF = mybir.ActivationFunctionType
ALU = mybir.AluOpType

NEG_SLOPE = 0.2

# Problem geometry (nn_Affinity_GAT_75557064671579)
N = 50000
E = 800000
F_IN = 128
H = 128
OUT = 64
NC_N = 8
NLOC = N // NC_N        # 6250 nodes per core
BLK = 125               # dst nodes per block
NB = NLOC // BLK        # 50 blocks per core
LO = 32768              # src table split (int16 index limit)
PAD_DST = 1000.0

LAST_EXEC_NS = [None]
LAST_RESULTS = [None]
DEBUG = False
DEBUG_LAYER = 1
NQUEUES = 4     # SWDGE queues used for gathers
MAXG = 8        # groups per gather instruction (1024 idxs; ring cap ~65/dma)
USE_TTR = False  # InstTensorTensorReduce wedges real HW; use stt + av add


def prep_inputs(x, edge_index, edge_attr):
    """Host-side graph prep: self loops, dst-shard, block layout, lo/hi
    src split, gather indices, one-hot S_T_aug."""
    src = edge_index[0].astype(np.int64)
    dst = edge_index[1].astype(np.int64)
    ea = edge_attr[:, 0].astype(np.float32)

    # add_self_loops with fill_value='mean' over incoming edges
    cnt = np.bincount(dst, minlength=N).astype(np.float32)
    asum = np.bincount(dst, weights=ea, minlength=N).astype(np.float32)
    loop_attr = (asum / np.maximum(cnt, 1.0)).astype(np.float32)
    src_all = np.concatenate([src, np.arange(N, dtype=np.int64)])
    dst_all = np.concatenate([dst, np.arange(N, dtype=np.int64)])
    ea_all = np.concatenate([ea, loop_attr])

    core_of = dst_all // NLOC
    percore = []
    nlo = np.zeros((NC_N, NB), np.int64)
    nhi = np.zeros((NC_N, NB), np.int64)
    for c in range(NC_N):
        m = core_of == c
        s_c = src_all[m]
        d_c = dst_all[m] - c * NLOC
        a_c = ea_all[m]
        b_c = d_c // BLK
        islo = s_c < LO
        key = b_c * 2 + (~islo).astype(np.int64)
        order = np.argsort(key, kind='stable')
        s_c, d_c, a_c, b_c, islo = (s_c[order], d_c[order], a_c[order],
                                    b_c[order], islo[order])
        percore.append((s_c, d_c, a_c, b_c, islo))
        nlo[c] = np.bincount(b_c[islo], minlength=NB)
        nhi[c] = np.bincount(b_c[~islo], minlength=NB)

    g_lo = ((nlo.max(axis=0) + 127) // 128).astype(np.int64)
    g_hi = ((nhi.max(axis=0) + 127) // 128).astype(np.int64)
    g_all = g_lo + g_hi
    g_start = np.concatenate([[0], np.cumsum(g_all)])
    g_total = int(g_start[-1])

    core_inputs = []
    for c in range(NC_N):
        s_c, d_c, a_c, b_c, islo = percore[c]
        idx16 = np.zeros((128, 8 * g_total), np.int16)
        dstc = np.full((128, g_total), PAD_DST, np.float32)
        ST = np.zeros((128, 128 * g_total), np.float32)

        # slot index per edge: block base + lo slot / hi region slot
        off_l = np.zeros(NB, np.int64)
        off_h = np.zeros(NB, np.int64)
        slot = np.zeros(len(s_c), np.int64)
        # edges are sorted by (block, hi) so within each (block, half)
        # they are consecutive
        for b in range(NB):
            lo_n = int(nlo[c, b])
            hi_n = int(nhi[c, b])
            base = g_start[b] * 128
            sel_lo = (b_c == b) & islo
            sel_hi = (b_c == b) & ~islo
            slot[sel_lo] = base + np.arange(lo_n)
            slot[sel_hi] = base + 128 * g_lo[b] + np.arange(hi_n)

        din = (d_c % BLK).astype(np.int64)
        ST[din, slot] = 1.0
        ST[126, slot] = 1.0
        ST[127, slot] = a_c
        dstc[slot % 128, slot // 128] = din.astype(np.float32)
        # Q7 cpu pair for queue q reads idxs from partitions [32q, 32q+32):
        # replicate the [16, cols] block across all 128 partitions.
        idxval = np.where(islo, s_c, s_c - LO).astype(np.int16)
        idx16[slot % 16, slot // 16] = idxval
        idx16 = np.tile(idx16[0:16], (8, 1))

        xT = np.ascontiguousarray(
            x[c * NLOC:(c + 1) * NLOC].T).astype(BF16)  # [F_IN, NLOC]
        core_inputs.append(dict(
            xT_slice=xT,
            idx_d=idx16,
            dstc_d=dstc,
            ST_d=ST.astype(BF16),
        ))
    meta = (tuple(g_lo.tolist()), tuple(g_hi.tolist()))
    return core_inputs, g_lo, g_hi, g_start, g_total, meta


def build_consts(Wl1, Wr1, We1, att1, b1, Wl2, Wr2, We2, att2, b2):
    att1 = np.asarray(att1, np.float32)
    att2 = np.asarray(att2, np.float32)
    Wl1 = np.asarray(Wl1, np.float32)
    Wr1 = np.asarray(Wr1, np.float32)
    Wl2 = np.asarray(Wl2, np.float32)
    Wr2 = np.asarray(Wr2, np.float32)
    We1 = np.asarray(We1, np.float32).reshape(-1)   # [H]
    We2 = np.asarray(We2, np.float32).reshape(-1)   # [OUT]

    def aug(W, att):
        return np.concatenate([W, NEG_SLOPE * (W @ att)[:, None]],
                              axis=1).astype(BF16)

    c2 = -(Wl2.sum(axis=0) + Wr2.sum(axis=0))       # [OUT]
    we1row = np.concatenate([We1, [NEG_SLOPE * float(We1 @ att1)]])
    we2row = np.concatenate([We2, [NEG_SLOPE * float(We2 @ att2)]])
    c2row = np.concatenate([c2, [NEG_SLOPE * float(c2 @ att2)]])

    return dict(
        Wl1aug=aug(Wl1, att1), Wr1aug=aug(Wr1, att1),
        Wl2aug=aug(Wl2, att2), Wr2aug=aug(Wr2, att2),
        We1rep=np.tile(we1row[None, :], (1, NB)).astype(BF16),
        We2rep=np.tile(we2row[None, :], (1, NB)).astype(BF16),
        Cr2rep=np.tile(c2row[None, :], (1, NB)).astype(BF16),
        att1t=np.tile(att1[None, :], (128, 1)).astype(BF16),
        att2t=np.tile(att2[None, :], (128, 1)).astype(BF16),
        b1t=np.tile(np.asarray(b1, np.float32)[None, :], (128, 1)),
        # U2 accumulates raw h@Wl2 rows (each +colsum(Wl2) vs (h-1)@Wl2);
        # U/s is a convex combination, so subtract it once via the bias.
        b2t=np.tile((np.asarray(b2, np.float32) - Wl2.sum(axis=0))[None, :],
                    (128, 1)),
        iota_row=np.tile(np.arange(BLK, dtype=np.float32)[None, :],
                         (128, 1)).astype(BF16),
        ident=np.eye(128, dtype=np.float32).astype(BF16),
        ones_col=np.ones((128, 1), np.float32).astype(BF16),
    )


def build_program(g_lo, g_hi, g_start, g_total):
    nc = bacc.Bacc("TRN2", target_bir_lowering=False, debug=False,
                   num_devices=NC_N, num_swdge_queues=NQUEUES)

    T1, T2 = 256, 128           # gather table row widths (cols)
    FA1, FA2 = H + 1, OUT + 1   # v columns incl. att.v accumulator
    CH1, CH2 = 3, 7             # psum-bank groups per chunk (<=512 f32)

    # ---- DRAM I/O ----
    xT_d = nc.dram_tensor("xT_slice", [F_IN, NLOC], BF, kind="ExternalInput")
    idx_d = nc.dram_tensor("idx_d", [128, 8 * g_total], I16, kind="ExternalInput")
    dstc_d = nc.dram_tensor("dstc_d", [128, g_total], F32, kind="ExternalInput")
    ST_d = nc.dram_tensor("ST_d", [128, 128 * g_total], BF, kind="ExternalInput")
    Wl1_d = nc.dram_tensor("Wl1aug", [F_IN, FA1], BF, kind="ExternalInput")
    Wr1_d = nc.dram_tensor("Wr1aug", [F_IN, FA1], BF, kind="ExternalInput")
    Wl2_d = nc.dram_tensor("Wl2aug", [H, FA2], BF, kind="ExternalInput")
    Wr2_d = nc.dram_tensor("Wr2aug", [H, FA2], BF, kind="ExternalInput")
    We1_d = nc.dram_tensor("We1rep", [1, NB * FA1], BF, kind="ExternalInput")
    We2_d = nc.dram_tensor("We2rep", [1, NB * FA2], BF, kind="ExternalInput")
    Cr2_d = nc.dram_tensor("Cr2rep", [1, NB * FA2], BF, kind="ExternalInput")
    att1_d = nc.dram_tensor("att1t", [128, H], BF, kind="ExternalInput")
    att2_d = nc.dram_tensor("att2t", [128, OUT], BF, kind="ExternalInput")
    b1_d = nc.dram_tensor("b1t", [128, H], F32, kind="ExternalInput")
    b2_d = nc.dram_tensor("b2t", [128, OUT], F32, kind="ExternalInput")
    iota_d = nc.dram_tensor("iota_row", [128, BLK], BF, kind="ExternalInput")
    ident_d = nc.dram_tensor("ident", [128, 128], BF, kind="ExternalInput")
    ones_d = nc.dram_tensor("ones_col", [128, 1], BF, kind="ExternalInput")
    out_d = nc.dram_tensor("out_slice", [NLOC, OUT], F32, kind="ExternalOutput")

    if DEBUG:
        logit1_dbg = nc.dram_tensor("logit1_dbg", [128, g_total], F32,
                                    kind="ExternalOutput")
        h_dbg = nc.dram_tensor("h_dbg", [NLOC, H], BF, kind="ExternalOutput")
        us_dbg = nc.dram_tensor("us_dbg", [NLOC, H + 1], F32,
                                kind="ExternalOutput")

    ag1_d = nc.dram_tensor("ag1_in", [NLOC, T1], BF)
    xl1_d = nc.dram_tensor("xl1_full", [N, T1], BF, addr_space="Shared")
    ag2_d = nc.dram_tensor("ag2_in", [NLOC, T2], BF)
    xl2_d = nc.dram_tensor("xl2_full", [N, T2], BF, addr_space="Shared")

    qn = [0]

    with tile.TileContext(nc) as tc:
        with ExitStack() as ctx:
            cpool = ctx.enter_context(tc.tile_pool(name="const", bufs=1))
            mm_pool = ctx.enter_context(tc.tile_pool(name="mmx", bufs=3))
            st_pool = ctx.enter_context(tc.tile_pool(name="stp", bufs=3))
            xlg_pool = ctx.enter_context(tc.tile_pool(name="xlg", bufs=3))
            idx_pool = ctx.enter_context(tc.tile_pool(name="idxp", bufs=3))
            sm_pool = ctx.enter_context(tc.tile_pool(name="small", bufs=4))
            r_pool = ctx.enter_context(tc.tile_pool(name="rz", bufs=4))
            scr_pool = ctx.enter_context(tc.tile_pool(name="scr", bufs=6))
            sw_pool = ctx.enter_context(tc.tile_pool(name="sw", bufs=6))
            np_pool = ctx.enter_context(tc.tile_pool(name="nodep", bufs=4))
            ps_mm = ctx.enter_context(tc.tile_pool(name="psmm", bufs=2, space="PSUM"))
            ps_v = ctx.enter_context(tc.tile_pool(name="psv", bufs=3, space="PSUM"))
            ps_u = ctx.enter_context(tc.tile_pool(name="psu", bufs=2, space="PSUM"))
            ps_t = ctx.enter_context(tc.tile_pool(name="pst", bufs=1, space="PSUM"))

            def cload(name, dram, shape, dt=BF):
                t = cpool.tile(shape, dt, tag=name)
                nc.sync.dma_start(out=t[:], in_=dram[:])
                return t

            xT_sb = cload("xT", xT_d, [F_IN, NLOC])
            Wl1_sb = cload("Wl1", Wl1_d, [F_IN, FA1])
            Wr1_sb = cload("Wr1", Wr1_d, [F_IN, FA1])
            Wl2_sb = cload("Wl2", Wl2_d, [H, FA2])
            Wr2_sb = cload("Wr2", Wr2_d, [H, FA2])
            att1_sb = cload("att1", att1_d, [128, H])
            att2_sb = cload("att2", att2_d, [128, OUT])
            b1_sb = cload("b1", b1_d, [128, H], F32)
            b2_sb = cload("b2", b2_d, [128, OUT], F32)
            iota_sb = cload("iota", iota_d, [128, BLK])
            ident_sb = cload("ident", ident_d, [128, 128])
            ones_sb = cload("ones", ones_d, [128, 1])

            hT_sb = cpool.tile([H, NLOC], BF, tag="hT")
            xr1_sb = cpool.tile([128, NB * FA1], BF, tag="xr1")
            xr2_sb = cpool.tile([128, NB * FA2], BF, tag="xr2")
            nc.vector.memset(xr1_sb[96:128, :], 0.0)
            nc.vector.memset(xr2_sb[96:128, :], 0.0)
            nc.sync.dma_start(out=xr1_sb[127:128, :], in_=We1_d[:])
            nc.sync.dma_start(out=xr2_sb[126:127, :], in_=Cr2_d[:])
            nc.sync.dma_start(out=xr2_sb[127:128, :], in_=We2_d[:])

            # tail columns of the gather-table rows: col FA (right after
            # the a_l column) is a constant 1.0 so one U-matmul also
            # accumulates s = sum(w) in psum col F+1; the rest are zero.
            zt = cpool.tile([BLK, T1 - FA1], BF, tag="zt")
            nc.vector.memset(zt[:], 0.0)
            nc.vector.memset(zt[:, 0:1], 1.0)
            for j in range(NB):
                rows = slice(j * BLK, (j + 1) * BLK)
                nc.sync.dma_start(out=ag1_d[rows, FA1:T1],
                                  in_=zt[:, 0:T1 - FA1])
                nc.sync.dma_start(out=ag2_d[rows, FA2:T2],
                                  in_=zt[:, 0:T2 - FA2])

            # ================= layer-1 matmul phase =================
            for j in range(NB):
                lhsT = xT_sb[:, j * BLK:(j + 1) * BLK]
                pa = ps_mm.tile([BLK, FA1], F32, tag="mm")
                nc.tensor.matmul(out=pa[:], lhsT=lhsT, rhs=Wl1_sb[:],
                                 start=True, stop=True)
                agt = mm_pool.tile([BLK, FA1], BF, tag="agt")
                nc.scalar.activation(agt[:], pa[:], AF.Copy)
                nc.sync.dma_start(out=ag1_d[j * BLK:(j + 1) * BLK, 0:FA1],
                                  in_=agt[:])
                pb = ps_mm.tile([BLK, FA1], F32, tag="mm")
                nc.tensor.matmul(out=pb[:], lhsT=lhsT, rhs=Wr1_sb[:],
                                 start=True, stop=True)
                nc.scalar.activation(xr1_sb[0:BLK, j * FA1:(j + 1) * FA1],
                                     pb[:], AF.Copy)

            tc.strict_bb_all_engine_barrier()
            if NC_N == 1:
                nc.sync.dma_start(out=xl1_d[:], in_=ag1_d[:])
            else:
                nc.gpsimd.collective_compute(
                    "AllGather", ALU.bypass,
                    replica_groups=[list(range(NC_N))],
                    ins=[ag1_d[:]], outs=[xl1_d[:]])
            tc.strict_bb_all_engine_barrier()

            def edge_layer(F, FA, T, CH, xl_full, xr_sb, att_sb, b_sb, sink,
                           dbg=False):
                for b in range(NB):
                    GL, GH = int(g_lo[b]), int(g_hi[b])
                    G = GL + GH
                    gs = int(g_start[b])
                    ST_t = st_pool.tile([128, 128 * G], BF, tag="st")
                    nc.sync.dma_start(
                        out=ST_t[:],
                        in_=ST_d[:, 128 * gs:128 * (gs + G)])
                    idx_t = idx_pool.tile([128, 8 * G], I16, tag="idx")
                    nc.sync.dma_start(
                        out=idx_t[:], in_=idx_d[:, 8 * gs:8 * (gs + G)])
                    dst_t = sm_pool.tile([128, G], F32, tag="dst")
                    nc.sync.dma_start(
                        out=dst_t[:], in_=dstc_d[:, gs:gs + G])

                    xlg = xlg_pool.tile([128, G, T], BF, tag="xlg")
                    for base, gn, tab in ((0, GL, xl_full[0:LO, :]),
                                          (GL, GH, xl_full[LO:N, :])):
                        for q0 in range(0, gn, MAXG):
                            n = min(MAXG, gn - q0)
                            o = base + q0
                            nc.gpsimd.dma_gather(
                                out_ap=xlg[:, o:o + n, :], in_ap=tab,
                                idxs_ap=idx_t[:, 8 * o:8 * (o + n)],
                                num_idxs=128 * n, num_idxs_reg=128 * n,
                                elem_size=T, queue_num=qn[0] % NQUEUES)
                            qn[0] += 1

                    logit_t = sm_pool.tile([128, G], F32, tag="logit")
                    if not USE_TTR:
                        lacc_t = sm_pool.tile([128, G], F32, tag="lacc")
                    xr_b = xr_sb[:, b * FA:(b + 1) * FA]
                    for c0 in range(0, G, CH):
                        ng = min(CH, G - c0)
                        pv = ps_v.tile([128, CH * FA], F32, tag="pv")
                        for gi in range(ng):
                            g = c0 + gi
                            sl = slice(gi * FA, (gi + 1) * FA)
                            nc.tensor.matmul(
                                out=pv[:, sl],
                                lhsT=ST_t[:, g * 128:(g + 1) * 128],
                                rhs=xr_b, start=True, stop=False)
                            nc.tensor.matmul(
                                out=pv[:, sl], lhsT=ident_sb[:],
                                rhs=xlg[:, g, 0:FA], start=False, stop=True)
                        r_t = r_pool.tile([128, CH * FA], BF, tag="r")
                        nc.scalar.activation(r_t[:, 0:ng * FA],
                                             pv[:, 0:ng * FA], AF.Relu,
                                             scale=1.0 - NEG_SLOPE)
                        for gi in range(ng):
                            g = c0 + gi
                            scr = scr_pool.tile([128, F], BF, tag="scr")
                            if USE_TTR:
                                nc.vector.tensor_tensor_reduce(
                                    out=scr[:],
                                    in0=r_t[:, gi * FA:gi * FA + F],
                                    in1=att_sb[:, 0:F],
                                    scale=1.0,
                                    scalar=pv[:, gi * FA + F:(gi + 1) * FA],
                                    op0=ALU.mult, op1=ALU.add,
                                    accum_out=logit_t[:, g:g + 1])
                            else:
                                # att.r into logit, then add the 0.2*att.v
                                # column on ACT (reads psum col F)
                                nc.vector.scalar_tensor_tensor(
                                    out=scr[:],
                                    in0=r_t[:, gi * FA:gi * FA + F],
                                    scalar=1.0, in1=att_sb[:, 0:F],
                                    op0=ALU.mult, op1=ALU.mult,
                                    accum_out=lacc_t[:, g:g + 1])
                        if not USE_TTR:
                            nc.vector.tensor_tensor(
                                out=logit_t[:, c0:c0 + ng],
                                in0=lacc_t[:, c0:c0 + ng],
                                in1=pv[:, FA - 1:ng * FA:FA],
                                op=ALU.add)

                    if DEBUG and dbg:
                        nc.sync.dma_start(out=logit1_dbg[:, gs:gs + G],
                                          in_=logit_t[:])
                    w_t = sm_pool.tile([128, G], F32, tag="w")
                    nc.scalar.activation(w_t[:], logit_t[:], AF.Exp)

                    # one matmul per group: rhs cols [xl(F) | a_l | 1.0]
                    # -> psum [U(F) | junk | s]
                    pU = ps_u.tile([BLK, F + 2], F32, tag="pU")
                    for g in range(G):
                        sw = sw_pool.tile([128, BLK], BF, tag="sw")
                        nc.vector.tensor_scalar(
                            out=sw[:], in0=iota_sb[:],
                            scalar1=dst_t[:, g:g + 1],
                            scalar2=w_t[:, g:g + 1],
                            op0=ALU.is_equal, op1=ALU.mult)
                        nc.tensor.matmul(out=pU[:], lhsT=sw[:, 0:BLK],
                                         rhs=xlg[:, g, 0:F + 2],
                                         start=(g == 0), stop=(g == G - 1))

                    if DEBUG and dbg:
                        ud = np_pool.tile([BLK, F + 1], F32, tag="ud")
                        nc.vector.tensor_scalar_add(ud[:, 0:F], pU[:, 0:F], 0.0)
                        nc.vector.tensor_scalar_add(
                            ud[:, F:F + 1], pU[:, F + 1:F + 2], 0.0)
                        nc.sync.dma_start(
                            out=us_dbg[b * BLK:(b + 1) * BLK, 0:F + 1],
                            in_=ud[:])
                    rs = sm_pool.tile([BLK, 1], F32, tag="rs")
                    nc.vector.reciprocal(rs[:], pU[:, F + 1:F + 2])
                    y = np_pool.tile([BLK, F], F32, tag="y")
                    nc.vector.scalar_tensor_tensor(
                        out=y[:], in0=pU[:, 0:F], scalar=rs[:],
                        in1=b_sb[0:BLK, 0:F],
                        op0=ALU.mult, op1=ALU.add)
                    mn = np_pool.tile([BLK, F], F32, tag="mn")
                    nc.vector.tensor_scalar_min(mn[:], y[:], 0.0)
                    exm = np_pool.tile([BLK, F], F32, tag="exm")
                    nc.scalar.activation(exm[:], mn[:], AF.Exp)
                    sink(b, y, exm)

            # ================= layer-1 edge phase =================
            def sink1(b, y, exm):
                # h = elu(y)+1 = max(y,0) + exp(min(y,0)); -1 folded into L2
                h = np_pool.tile([BLK, H], BF, tag="h")
                nc.vector.scalar_tensor_tensor(
                    out=h[:], in0=y[:], scalar=0.0, in1=exm[:],
                    op0=ALU.max, op1=ALU.add)
                pT = ps_t.tile([H, BLK], F32, tag="pT")
                nc.tensor.matmul(out=pT[:], lhsT=h[:],
                                 rhs=ident_sb[0:BLK, 0:BLK],
                                 start=True, stop=True)
                nc.scalar.activation(hT_sb[:, b * BLK:(b + 1) * BLK],
                                     pT[:], AF.Copy)
                if DEBUG:
                    nc.sync.dma_start(out=h_dbg[b * BLK:(b + 1) * BLK, :],
                                      in_=h[:])

            edge_layer(H, FA1, T1, CH1, xl1_d, xr1_sb, att1_sb, b1_sb, sink1,
                       dbg=DEBUG_LAYER == 1)

            # ================= layer-2 matmul phase =================
            for j in range(NB):
                lhsT = hT_sb[:, j * BLK:(j + 1) * BLK]
                pa = ps_mm.tile([BLK, FA2], F32, tag="mm")
                nc.tensor.matmul(out=pa[:], lhsT=lhsT, rhs=Wl2_sb[:],
                                 start=True, stop=True)
                agt = mm_pool.tile([BLK, FA2], BF, tag="agt2")
                nc.scalar.activation(agt[:], pa[:], AF.Copy)
                nc.sync.dma_start(out=ag2_d[j * BLK:(j + 1) * BLK, 0:FA2],
                                  in_=agt[:])
                pb = ps_mm.tile([BLK, FA2], F32, tag="mm")
                nc.tensor.matmul(out=pb[:], lhsT=lhsT, rhs=Wr2_sb[:],
                                 start=True, stop=True)
                nc.scalar.activation(xr2_sb[0:BLK, j * FA2:(j + 1) * FA2],
                                     pb[:], AF.Copy)

            tc.strict_bb_all_engine_barrier()
            if NC_N == 1:
                nc.sync.dma_start(out=xl2_d[:], in_=ag2_d[:])
            else:
                nc.gpsimd.collective_compute(
                    "AllGather", ALU.bypass,
                    replica_groups=[list(range(NC_N))],
                    ins=[ag2_d[:]], outs=[xl2_d[:]])
            tc.strict_bb_all_engine_barrier()

            # ================= layer-2 edge phase =================
            def sink2(b, y, exm):
                # out = elu(y) = (y - min(y,0)) + (exp(min(y,0)) - 1)
                mn2 = np_pool.tile([BLK, OUT], F32, tag="ymn")
                nc.vector.scalar_tensor_tensor(
                    out=mn2[:], in0=y[:], scalar=0.0, in1=exm[:],
                    op0=ALU.max, op1=ALU.add)
                h2 = np_pool.tile([BLK, OUT], F32, tag="h2")
                nc.vector.tensor_scalar_add(h2[:], mn2[:], -1.0)
                nc.sync.dma_start(out=out_d[b * BLK:(b + 1) * BLK, :],
                                  in_=h2[:])

            edge_layer(OUT, FA2, T2, CH2, xl2_d, xr2_sb, att2_sb, b2_sb, sink2,
                       dbg=DEBUG_LAYER == 2)

    nc.compile()
    return nc


_CACHE = {}


def kernel(x, edge_index, edge_attr, Wl1, Wr1, We1, att1, b1,
           Wl2, Wr2, We2, att2, b2, _trace=False):
    from concourse.bass_utils import run_bass_kernel_spmd

    x = np.asarray(x, np.float32)
    edge_index = np.asarray(edge_index)
    edge_attr = np.asarray(edge_attr, np.float32)

    core_inputs, g_lo, g_hi, g_start, g_total, meta = prep_inputs(
        x, edge_index, edge_attr)

    if meta not in _CACHE:
        _CACHE[meta] = build_program(g_lo, g_hi, g_start, g_total)
    nc = _CACHE[meta]

    consts = build_consts(Wl1, Wr1, We1, att1, b1, Wl2, Wr2, We2, att2, b2)
    in_maps = []
    for c in range(NC_N):
        m = dict(consts)
        m.update(core_inputs[c])
        in_maps.append(m)

    res = run_bass_kernel_spmd(nc, in_maps, list(range(NC_N)), trace=_trace)
    LAST_EXEC_NS[0] = res.exec_time_ns
    LAST_RESULTS[0] = res.results
    out = np.concatenate([res.results[c]["out_slice"] for c in range(NC_N)],
                         axis=0)
    return out.astype(np.float32)


# revision 49
# speedup vs baseline: 1.1707x; 1.0242x over previous
"""Two-layer GATv2 (heads=1) on 8 Trainium2 NeuronCores.

Sharding: nodes partitioned by dst across 8 cores (6250 each), dst-blocks of
125 nodes. Edges of a block are sorted lo/hi by src (split at 32768 so
indices fit int16) and processed in groups of 128.

Per layer:
  - matmul phase: xl = x@[Wl | 0.2*Wl@att], xr likewise (PE, bf16); xl rows
    AllGathered into a [N, T] row table (T = 256/128 cols, 512/256B rows).
  - edge phase per dst-block:
      xlg  <- batched dma_gather of xl rows (two gathers: lo/hi table halves)
      psum <- S_T_aug.T @ xr_aug  (one-hot dst select + We*ea + const row)
            + I.T @ xlg           (adds xl[src]; col F accumulates 0.2*att.v)
      r    <- Relu(0.8*psum) on ACT      (lrelu(v) = 0.2v + relu(0.8v))
      logit<- ttr(r, att, mult, add, init=psum_col_F)  -> 0.2*att.v + att.r
      w    <- Exp(logit)
      sw   <- (iota==dst)*w (bf16 one-hot), U += sw.T@xlg, s += sw.T@1 (PE)
      h    <- elu(U/s + b) (+1 folded into next layer's const row)
Weights replicated; layer-2 constant correction (elu's -1) folded into ST
row 126 x xr row 126.
"""
import sys

sys.path.insert(0, '/opt/trn_rl_repo')

import numpy as np
import ml_dtypes
from contextlib import ExitStack

import concourse.bass as bass
import concourse.tile as tile
import concourse.bacc as bacc
from concourse import mybir

BF16 = ml_dtypes.bfloat16
F32 = mybir.dt.float32
BF = mybir.dt.bfloat16
I16 = mybir.dt.int16
AF = mybir.ActivationFunctionType
ALU = mybir.AluOpType

NEG_SLOPE = 0.2

# Problem geometry (nn_Affinity_GAT_75557064671579)
N = 50000
E = 800000
F_IN = 128
H = 128
OUT = 64
NC_N = 8
NLOC = N // NC_N        # 6250 nodes per core
BLK = 125               # dst nodes per block
NB = NLOC // BLK        # 50 blocks per core
LO = 32768              # src table split (int16 index limit)
PAD_DST = 1000.0

LAST_EXEC_NS = [None]
LAST_RESULTS = [None]
DEBUG = False
DEBUG_LAYER = 1
NQUEUES = 4     # SWDGE queues used for gathers
MAXG = 8        # groups per gather instruction (1024 idxs; ring cap ~65/dma)
USE_TTR = False  # InstTensorTensorReduce wedges real HW; use stt + av add


def prep_inputs(x, edge_index, edge_attr):
    """Host-side graph prep: self loops, dst-shard, block layout, lo/hi
    src split, gather indices, one-hot S_T_aug."""
    src = edge_index[0].astype(np.int64)
    dst = edge_index[1].astype(np.int64)
    ea = edge_attr[:, 0].astype(np.float32)

    # add_self_loops with fill_value='mean' over incoming edges
    cnt = np.bincount(dst, minlength=N).astype(np.float32)
    asum = np.bincount(dst, weights=ea, minlength=N).astype(np.float32)
    loop_attr = (asum / np.maximum(cnt, 1.0)).astype(np.float32)
    src_all = np.concatenate([src, np.arange(N, dtype=np.int64)])
    dst_all = np.concatenate([dst, np.arange(N, dtype=np.int64)])
    ea_all = np.concatenate([ea, loop_attr])

    core_of = dst_all // NLOC
    percore = []
    nlo = np.zeros((NC_N, NB), np.int64)
    nhi = np.zeros((NC_N, NB), np.int64)
    for c in range(NC_N):
        m = core_of == c
        s_c = src_all[m]
        d_c = dst_all[m] - c * NLOC
        a_c = ea_all[m]
        b_c = d_c // BLK
        islo = s_c < LO
        key = b_c * 2 + (~islo).astype(np.int64)
        order = np.argsort(key, kind='stable')
        s_c, d_c, a_c, b_c, islo = (s_c[order], d_c[order], a_c[order],
                                    b_c[order], islo[order])
        percore.append((s_c, d_c, a_c, b_c, islo))
        nlo[c] = np.bincount(b_c[islo], minlength=NB)
        nhi[c] = np.bincount(b_c[~islo], minlength=NB)

    g_lo = ((nlo.max(axis=0) + 127) // 128).astype(np.int64)
    g_hi = ((nhi.max(axis=0) + 127) // 128).astype(np.int64)
    g_all = g_lo + g_hi
    g_start = np.concatenate([[0], np.cumsum(g_all)])
    g_total = int(g_start[-1])

    core_inputs = []
    for c in range(NC_N):
        s_c, d_c, a_c, b_c, islo = percore[c]
        idx16 = np.zeros((128, 8 * g_total), np.int16)
        dstc = np.full((128, g_total), PAD_DST, np.float32)
        ST = np.zeros((128, 128 * g_total), np.float32)

        # slot index per edge: block base + lo slot / hi region slot
        off_l = np.zeros(NB, np.int64)
        off_h = np.zeros(NB, np.int64)
        slot = np.zeros(len(s_c), np.int64)
        # edges are sorted by (block, hi) so within each (block, half)
        # they are consecutive
        for b in range(NB):
            lo_n = int(nlo[c, b])
            hi_n = int(nhi[c, b])
            base = g_start[b] * 128
            sel_lo = (b_c == b) & islo
            sel_hi = (b_c == b) & ~islo
            slot[sel_lo] = base + np.arange(lo_n)
            slot[sel_hi] = base + 128 * g_lo[b] + np.arange(hi_n)

        din = (d_c % BLK).astype(np.int64)
        ST[din, slot] = 1.0
        ST[126, slot] = 1.0
        ST[127, slot] = a_c
        dstc[slot % 128, slot // 128] = din.astype(np.float32)
        # Q7 cpu pair for queue q reads idxs from partitions [32q, 32q+32):
        # replicate the [16, cols] block across all 128 partitions.
        idxval = np.where(islo, s_c, s_c - LO).astype(np.int16)
        idx16[slot % 16, slot // 16] = idxval
        idx16 = np.tile(idx16[0:16], (8, 1))

        xT = np.ascontiguousarray(
            x[c * NLOC:(c + 1) * NLOC].T).astype(BF16)  # [F_IN, NLOC]
        core_inputs.append(dict(
            xT_slice=xT,
            idx_d=idx16,
            dstc_d=dstc,
            ST_d=ST.astype(BF16),
        ))
    meta = (tuple(g_lo.tolist()), tuple(g_hi.tolist()))
    return core_inputs, g_lo, g_hi, g_start, g_total, meta


def build_consts(Wl1, Wr1, We1, att1, b1, Wl2, Wr2, We2, att2, b2):
    att1 = np.asarray(att1, np.float32)
    att2 = np.asarray(att2, np.float32)
    Wl1 = np.asarray(Wl1, np.float32)
    Wr1 = np.asarray(Wr1, np.float32)
    Wl2 = np.asarray(Wl2, np.float32)
    Wr2 = np.asarray(Wr2, np.float32)
    We1 = np.asarray(We1, np.float32).reshape(-1)   # [H]
    We2 = np.asarray(We2, np.float32).reshape(-1)   # [OUT]

    def aug(W, att):
        return np.concatenate([W, NEG_SLOPE * (W @ att)[:, None]],
                              axis=1).astype(BF16)

    c2 = -(Wl2.sum(axis=0) + Wr2.sum(axis=0))       # [OUT]
    we1row = np.concatenate([We1, [NEG_SLOPE * float(We1 @ att1)]])
    we2row = np.concatenate([We2, [NEG_SLOPE * float(We2 @ att2)]])
    c2row = np.concatenate([c2, [NEG_SLOPE * float(c2 @ att2)]])

    return dict(
        Wl1aug=aug(Wl1, att1), Wr1aug=aug(Wr1, att1),
        Wl2aug=aug(Wl2, att2), Wr2aug=aug(Wr2, att2),
        We1rep=np.tile(we1row[None, :], (1, NB)).astype(BF16),
        We2rep=np.tile(we2row[None, :], (1, NB)).astype(BF16),
        Cr2rep=np.tile(c2row[None, :], (1, NB)).astype(BF16),
        att1t=np.tile(att1[None, :], (128, 1)).astype(BF16),
        att2t=np.tile(att2[None, :], (128, 1)).astype(BF16),
        b1t=np.tile(np.asarray(b1, np.float32)[None, :], (128, 1)),
        # U2 accumulates raw h@Wl2 rows (each +colsum(Wl2) vs (h-1)@Wl2);
        # U/s is a convex combination, so subtract it once via the bias.
        b2t=np.tile((np.asarray(b2, np.float32) - Wl2.sum(axis=0))[None, :],
                    (128, 1)),
        iota_row=np.tile(np.arange(BLK, dtype=np.float32)[None, :],
                         (128, 1)).astype(BF16),
        ident=np.eye(128, dtype=np.float32).astype(BF16),
        ones_col=np.ones((128, 1), np.float32).astype(BF16),
    )


def build_program(g_lo, g_hi, g_start, g_total):
    nc = bacc.Bacc("TRN2", target_bir_lowering=False, debug=False,
                   num_devices=NC_N, num_swdge_queues=NQUEUES)

    T1, T2 = 256, 128           # gather table row widths (cols)
    FA1, FA2 = H + 1, OUT + 1   # v columns incl. att.v accumulator
    CH1, CH2 = 3, 7             # psum-bank groups per chunk (<=512 f32)

    # ---- DRAM I/O ----
    xT_d = nc.dram_tensor("xT_slice", [F_IN, NLOC], BF, kind="ExternalInput")
    idx_d = nc.dram_tensor("idx_d", [128, 8 * g_total], I16, kind="ExternalInput")
    dstc_d = nc.dram_tensor("dstc_d", [128, g_total], F32, kind="ExternalInput")
    ST_d = nc.dram_tensor("ST_d", [128, 128 * g_total], BF, kind="ExternalInput")
    Wl1_d = nc.dram_tensor("Wl1aug", [F_IN, FA1], BF, kind="ExternalInput")
    Wr1_d = nc.dram_tensor("Wr1aug", [F_IN, FA1], BF, kind="ExternalInput")
    Wl2_d = nc.dram_tensor("Wl2aug", [H, FA2], BF, kind="ExternalInput")
    Wr2_d = nc.dram_tensor("Wr2aug", [H, FA2], BF, kind="ExternalInput")
    We1_d = nc.dram_tensor("We1rep", [1, NB * FA1], BF, kind="ExternalInput")
    We2_d = nc.dram_tensor("We2rep", [1, NB * FA2], BF, kind="ExternalInput")
    Cr2_d = nc.dram_tensor("Cr2rep", [1, NB * FA2], BF, kind="ExternalInput")
    att1_d = nc.dram_tensor("att1t", [128, H], BF, kind="ExternalInput")
    att2_d = nc.dram_tensor("att2t", [128, OUT], BF, kind="ExternalInput")
    b1_d = nc.dram_tensor("b1t", [128, H], F32, kind="ExternalInput")
    b2_d = nc.dram_tensor("b2t", [128, OUT], F32, kind="ExternalInput")
    iota_d = nc.dram_tensor("iota_row", [128, BLK], BF, kind="ExternalInput")
    ident_d = nc.dram_tensor("ident", [128, 128], BF, kind="ExternalInput")
    ones_d = nc.dram_tensor("ones_col", [128, 1], BF, kind="ExternalInput")
    out_d = nc.dram_tensor("out_slice", [NLOC, OUT], F32, kind="ExternalOutput")

    if DEBUG:
        logit1_dbg = nc.dram_tensor("logit1_dbg", [128, g_total], F32,
                                    kind="ExternalOutput")
        h_dbg = nc.dram_tensor("h_dbg", [NLOC, H], BF, kind="ExternalOutput")
        us_dbg = nc.dram_tensor("us_dbg", [NLOC, H + 1], F32,
                                kind="ExternalOutput")

    ag1_d = nc.dram_tensor("ag1_in", [NLOC, T1], BF)
    xl1_d = nc.dram_tensor("xl1_full", [N, T1], BF, addr_space="Shared")
    ag2_d = nc.dram_tensor("ag2_in", [NLOC, T2], BF)
    xl2_d = nc.dram_tensor("xl2_full", [N, T2], BF, addr_space="Shared")

    qn = [0]

    with tile.TileContext(nc) as tc:
        with ExitStack() as ctx:
            cpool = ctx.enter_context(tc.tile_pool(name="const", bufs=1))
            mm_pool = ctx.enter_context(tc.tile_pool(name="mmx", bufs=3))
            st_pool = ctx.enter_context(tc.tile_pool(name="stp", bufs=3))
            xlg_pool = ctx.enter_context(tc.tile_pool(name="xlg", bufs=3))
            idx_pool = ctx.enter_context(tc.tile_pool(name="idxp", bufs=3))
            sm_pool = ctx.enter_context(tc.tile_pool(name="small", bufs=4))
            r_pool = ctx.enter_context(tc.tile_pool(name="rz", bufs=4))
            scr_pool = ctx.enter_context(tc.tile_pool(name="scr", bufs=6))
            sw_pool = ctx.enter_context(tc.tile_pool(name="sw", bufs=6))
            np_pool = ctx.enter_context(tc.tile_pool(name="nodep", bufs=4))
            ps_mm = ctx.enter_context(tc.tile_pool(name="psmm", bufs=2, space="PSUM"))
            ps_v = ctx.enter_context(tc.tile_pool(name="psv", bufs=3, space="PSUM"))
            ps_u = ctx.enter_context(tc.tile_pool(name="psu", bufs=2, space="PSUM"))
            ps_t = ctx.enter_context(tc.tile_pool(name="pst", bufs=1, space="PSUM"))

            def cload(name, dram, shape, dt=BF):
                t = cpool.tile(shape, dt, tag=name)
                nc.sync.dma_start(out=t[:], in_=dram[:])
                return t

            xT_sb = cload("xT", xT_d, [F_IN, NLOC])
            Wl1_sb = cload("Wl1", Wl1_d, [F_IN, FA1])
            Wr1_sb = cload("Wr1", Wr1_d, [F_IN, FA1])
            Wl2_sb = cload("Wl2", Wl2_d, [H, FA2])
            Wr2_sb = cload("Wr2", Wr2_d, [H, FA2])
            att1_sb = cload("att1", att1_d, [128, H])
            att2_sb = cload("att2", att2_d, [128, OUT])
            b1_sb = cload("b1", b1_d, [128, H], F32)
            b2_sb = cload("b2", b2_d, [128, OUT], F32)
            iota_sb = cload("iota", iota_d, [128, BLK])
            ident_sb = cload("ident", ident_d, [128, 128])
            ones_sb = cload("ones", ones_d, [128, 1])

            hT_sb = cpool.tile([H, NLOC], BF, tag="hT")
            xr1_sb = cpool.tile([128, NB * FA1], BF, tag="xr1")
            xr2_sb = cpool.tile([128, NB * FA2], BF, tag="xr2")
            nc.vector.memset(xr1_sb[96:128, :], 0.0)
            nc.vector.memset(xr2_sb[96:128, :], 0.0)
            nc.sync.dma_start(out=xr1_sb[127:128, :], in_=We1_d[:])
            nc.sync.dma_start(out=xr2_sb[126:127, :], in_=Cr2_d[:])
            nc.sync.dma_start(out=xr2_sb[127:128, :], in_=We2_d[:])

            # tail columns of the gather-table rows: col FA (right after
            # the a_l column) is a constant 1.0 so one U-matmul also
            # accumulates s = sum(w) in psum col F+1; the rest are zero.
            zt = cpool.tile([BLK, T1 - FA1], BF, tag="zt")
            nc.vector.memset(zt[:], 0.0)
            nc.vector.memset(zt[:, 0:1], 1.0)
            for j in range(NB):
                rows = slice(j * BLK, (j + 1) * BLK)
                nc.sync.dma_start(out=ag1_d[rows, FA1:T1],
                                  in_=zt[:, 0:T1 - FA1])
                nc.sync.dma_start(out=ag2_d[rows, FA2:T2],
                                  in_=zt[:, 0:T2 - FA2])

            # ================= layer-1 matmul phase =================
            for j in range(NB):
                lhsT = xT_sb[:, j * BLK:(j + 1) * BLK]
                pa = ps_mm.tile([BLK, FA1], F32, tag="mm")
                nc.tensor.matmul(out=pa[:], lhsT=lhsT, rhs=Wl1_sb[:],
                                 start=True, stop=True)
                agt = mm_pool.tile([BLK, FA1], BF, tag="agt")
                nc.scalar.activation(agt[:], pa[:], AF.Copy)
                nc.sync.dma_start(out=ag1_d[j * BLK:(j + 1) * BLK, 0:FA1],
                                  in_=agt[:])
                pb = ps_mm.tile([BLK, FA1], F32, tag="mm")
                nc.tensor.matmul(out=pb[:], lhsT=lhsT, rhs=Wr1_sb[:],
                                 start=True, stop=True)
                nc.scalar.activation(xr1_sb[0:BLK, j * FA1:(j + 1) * FA1],
                                     pb[:], AF.Copy)

            tc.strict_bb_all_engine_barrier()
            if NC_N == 1:
                nc.sync.dma_start(out=xl1_d[:], in_=ag1_d[:])
            else:
                nc.gpsimd.collective_compute(
                    "AllGather", ALU.bypass,
                    replica_groups=[list(range(NC_N))],
                    ins=[ag1_d[:]], outs=[xl1_d[:]])
            tc.strict_bb_all_engine_barrier()

            def edge_layer(F, FA, T, CH, xl_full, xr_sb, att_sb, b_sb, sink,
                           dbg=False):
                for b in range(NB):
                    GL, GH = int(g_lo[b]), int(g_hi[b])
                    G = GL + GH
                    gs = int(g_start[b])
                    ST_t = st_pool.tile([128, 128 * G], BF, tag="st")
                    nc.sync.dma_start(
                        out=ST_t[:],
                        in_=ST_d[:, 128 * gs:128 * (gs + G)])
                    idx_t = idx_pool.tile([128, 8 * G], I16, tag="idx")
                    nc.sync.dma_start(
                        out=idx_t[:], in_=idx_d[:, 8 * gs:8 * (gs + G)])
                    dst_t = sm_pool.tile([128, G], F32, tag="dst")
                    nc.sync.dma_start(
                        out=dst_t[:], in_=dstc_d[:, gs:gs + G])

                    xlg = xlg_pool.tile([128, G, T], BF, tag="xlg")
                    for base, gn, tab in ((0, GL, xl_full[0:LO, :]),
                                          (GL, GH, xl_full[LO:N, :])):
                        for q0 in range(0, gn, MAXG):
                            n = min(MAXG, gn - q0)
                            o = base + q0
                            nc.gpsimd.dma_gather(
                                out_ap=xlg[:, o:o + n, :], in_ap=tab,
                                idxs_ap=idx_t[:, 8 * o:8 * (o + n)],
                                num_idxs=128 * n, num_idxs_reg=128 * n,
                                elem_size=T, queue_num=qn[0] % NQUEUES)
                            qn[0] += 1

                    logit_t = sm_pool.tile([128, G], F32, tag="logit")
                    w_t = sm_pool.tile([128, G], F32, tag="w")
                    if not USE_TTR:
                        lacc_t = sm_pool.tile([128, G], F32, tag="lacc")
                    xr_b = xr_sb[:, b * FA:(b + 1) * FA]
                    for c0 in range(0, G, CH):
                        ng = min(CH, G - c0)
                        pv = ps_v.tile([128, CH * FA], F32, tag="pv")
                        for gi in range(ng):
                            g = c0 + gi
                            sl = slice(gi * FA, (gi + 1) * FA)
                            nc.tensor.matmul(
                                out=pv[:, sl],
                                lhsT=ST_t[:, g * 128:(g + 1) * 128],
                                rhs=xr_b, start=True, stop=False)
                            nc.tensor.matmul(
                                out=pv[:, sl], lhsT=ident_sb[:],
                                rhs=xlg[:, g, 0:FA], start=False, stop=True)
                        r_t = r_pool.tile([128, CH * FA], BF, tag="r")
                        nc.scalar.activation(r_t[:, 0:ng * FA],
                                             pv[:, 0:ng * FA], AF.Relu,
                                             scale=1.0 - NEG_SLOPE)
                        for gi in range(ng):
                            g = c0 + gi
                            scr = scr_pool.tile([128, F], BF, tag="scr")
                            if USE_TTR:
                                nc.vector.tensor_tensor_reduce(
                                    out=scr[:],
                                    in0=r_t[:, gi * FA:gi * FA + F],
                                    in1=att_sb[:, 0:F],
                                    scale=1.0,
                                    scalar=pv[:, gi * FA + F:(gi + 1) * FA],
                                    op0=ALU.mult, op1=ALU.add,
                                    accum_out=logit_t[:, g:g + 1])
                            else:
                                # att.r into logit, then add the 0.2*att.v
                                # column on ACT (reads psum col F)
                                nc.vector.scalar_tensor_tensor(
                                    out=scr[:],
                                    in0=r_t[:, gi * FA:gi * FA + F],
                                    scalar=1.0, in1=att_sb[:, 0:F],
                                    op0=ALU.mult, op1=ALU.mult,
                                    accum_out=lacc_t[:, g:g + 1])
                        if not USE_TTR:
                            nc.vector.tensor_tensor(
                                out=logit_t[:, c0:c0 + ng],
                                in0=lacc_t[:, c0:c0 + ng],
                                in1=pv[:, FA - 1:ng * FA:FA],
                                op=ALU.add)
                        # per-chunk exp so the sw/U chain starts before the
                        # whole block's logits are done
                        nc.scalar.activation(w_t[:, c0:c0 + ng],
                                             logit_t[:, c0:c0 + ng], AF.Exp)

                    if DEBUG and dbg:
                        nc.sync.dma_start(out=logit1_dbg[:, gs:gs + G],
                                          in_=logit_t[:])

                    # one matmul per group: rhs cols [xl(F) | a_l | 1.0]
                    # -> psum [U(F) | junk | s]
                    pU = ps_u.tile([BLK, F + 2], F32, tag="pU")
                    for g in range(G):
                        sw = sw_pool.tile([128, BLK], BF, tag="sw")
                        nc.vector.tensor_scalar(
                            out=sw[:], in0=iota_sb[:],
                            scalar1=dst_t[:, g:g + 1],
                            scalar2=w_t[:, g:g + 1],
                            op0=ALU.is_equal, op1=ALU.mult)
                        nc.tensor.matmul(out=pU[:], lhsT=sw[:, 0:BLK],
                                         rhs=xlg[:, g, 0:F + 2],
                                         start=(g == 0), stop=(g == G - 1))

                    if DEBUG and dbg:
                        ud = np_pool.tile([BLK, F + 1], F32, tag="ud")
                        nc.vector.tensor_scalar_add(ud[:, 0:F], pU[:, 0:F], 0.0)
                        nc.vector.tensor_scalar_add(
                            ud[:, F:F + 1], pU[:, F + 1:F + 2], 0.0)
                        nc.sync.dma_start(
                            out=us_dbg[b * BLK:(b + 1) * BLK, 0:F + 1],
                            in_=ud[:])
                    rs = sm_pool.tile([BLK, 1], F32, tag="rs")
                    nc.vector.reciprocal(rs[:], pU[:, F + 1:F + 2])
                    y = np_pool.tile([BLK, F], F32, tag="y")
                    nc.vector.scalar_tensor_tensor(
                        out=y[:], in0=pU[:, 0:F], scalar=rs[:],
                        in1=b_sb[0:BLK, 0:F],
                        op0=ALU.mult, op1=ALU.add)
                    mn = np_pool.tile([BLK, F], F32, tag="mn")
                    nc.vector.tensor_scalar_min(mn[:], y[:], 0.0)
                    exm = np_pool.tile([BLK, F], F32, tag="exm")
                    nc.scalar.activation(exm[:], mn[:], AF.Exp)
                    sink(b, y, exm)

            # ================= layer-1 edge phase =================
            def sink1(b, y, exm):
                # h = elu(y)+1 = max(y,0) + exp(min(y,0)); -1 folded into L2
                h = np_pool.tile([BLK, H], BF, tag="h")
                nc.vector.scalar_tensor_tensor(
                    out=h[:], in0=y[:], scalar=0.0, in1=exm[:],
                    op0=ALU.max, op1=ALU.add)
                pT = ps_t.tile([H, BLK], F32, tag="pT")
                nc.tensor.matmul(out=pT[:], lhsT=h[:],
                                 rhs=ident_sb[0:BLK, 0:BLK],
                                 start=True, stop=True)
                nc.scalar.activation(hT_sb[:, b * BLK:(b + 1) * BLK],
                                     pT[:], AF.Copy)
                if DEBUG:
                    nc.sync.dma_start(out=h_dbg[b * BLK:(b + 1) * BLK, :],
                                      in_=h[:])

            edge_layer(H, FA1, T1, CH1, xl1_d, xr1_sb, att1_sb, b1_sb, sink1,
                       dbg=DEBUG_LAYER == 1)

            # ================= layer-2 matmul phase =================
            for j in range(NB):
                lhsT = hT_sb[:, j * BLK:(j + 1) * BLK]
                pa = ps_mm.tile([BLK, FA2], F32, tag="mm")
                nc.tensor.matmul(out=pa[:], lhsT=lhsT, rhs=Wl2_sb[:],
                                 start=True, stop=True)
                agt = mm_pool.tile([BLK, FA2], BF, tag="agt2")
                nc.scalar.activation(agt[:], pa[:], AF.Copy)
                nc.sync.dma_start(out=ag2_d[j * BLK:(j + 1) * BLK, 0:FA2],
                                  in_=agt[:])
                pb = ps_mm.tile([BLK, FA2], F32, tag="mm")
                nc.tensor.matmul(out=pb[:], lhsT=lhsT, rhs=Wr2_sb[:],
                                 start=True, stop=True)
                nc.scalar.activation(xr2_sb[0:BLK, j * FA2:(j + 1) * FA2],
                                     pb[:], AF.Copy)

            tc.strict_bb_all_engine_barrier()
            if NC_N == 1:
                nc.sync.dma_start(out=xl2_d[:], in_=ag2_d[:])
            else:
                nc.gpsimd.collective_compute(
                    "AllGather", ALU.bypass,
                    replica_groups=[list(range(NC_N))],
                    ins=[ag2_d[:]], outs=[xl2_d[:]])
            tc.strict_bb_all_engine_barrier()

            # ================= layer-2 edge phase =================
            def sink2(b, y, exm):
                # out = elu(y) = (y - min(y,0)) + (exp(min(y,0)) - 1)
                mn2 = np_pool.tile([BLK, OUT], F32, tag="ymn")
                nc.vector.scalar_tensor_tensor(
                    out=mn2[:], in0=y[:], scalar=0.0, in1=exm[:],
                    op0=ALU.max, op1=ALU.add)
                h2 = np_pool.tile([BLK, OUT], F32, tag="h2")
                nc.vector.tensor_scalar_add(h2[:], mn2[:], -1.0)
                nc.sync.dma_start(out=out_d[b * BLK:(b + 1) * BLK, :],
                                  in_=h2[:])

            edge_layer(OUT, FA2, T2, CH2, xl2_d, xr2_sb, att2_sb, b2_sb, sink2,
                       dbg=DEBUG_LAYER == 2)

    nc.compile()
    return nc


_CACHE = {}


def kernel(x, edge_index, edge_attr, Wl1, Wr1, We1, att1, b1,
           Wl2, Wr2, We2, att2, b2, _trace=False):
    from concourse.bass_utils import run_bass_kernel_spmd

    x = np.asarray(x, np.float32)
    edge_index = np.asarray(edge_index)
    edge_attr = np.asarray(edge_attr, np.float32)

    core_inputs, g_lo, g_hi, g_start, g_total, meta = prep_inputs(
        x, edge_index, edge_attr)

    if meta not in _CACHE:
        _CACHE[meta] = build_program(g_lo, g_hi, g_start, g_total)
    nc = _CACHE[meta]

    consts = build_consts(Wl1, Wr1, We1, att1, b1, Wl2, Wr2, We2, att2, b2)
    in_maps = []
    for c in range(NC_N):
        m = dict(consts)
        m.update(core_inputs[c])
        in_maps.append(m)

    res = run_bass_kernel_spmd(nc, in_maps, list(range(NC_N)), trace=_trace)
    LAST_EXEC_NS[0] = res.exec_time_ns
    LAST_RESULTS[0] = res.results
    out = np.concatenate([res.results[c]["out_slice"] for c in range(NC_N)],
                         axis=0)
    return out.astype(np.float32)
